# revision 8
# baseline (speedup 1.0000x reference)
"""Trainium2 Bass kernel for nn_BPModel: LSTM encoder -> latent ODE (RK4) -> decoder.

Data-parallel over 8 NeuronCores: batch 4096 -> 512 per core. All parameters
replicated. Everything stays on-chip (SBUF) in a transposed [feature, batch]
layout; matmuls run as fp32r (1 col/cycle at the PE when N>=256).

v2 redesign (vs v1 baseline at 1.85ms): the ACT (scalar) engine was the
bottleneck at 77% busy in the LSTM phase (8 sigmoid/tanh insts per step).
Key changes:

LSTM (T=256 steps, 2 interleaved half-batch streams of 256 cols each):
  - tanh eliminated via tanh(x) = 2*sigmoid(2x) - 1:
      * g-gate preactivation is pre-doubled (host scales the g columns of
        Wih/Whh/bias by 2), so one [128,1024] Sigmoid covers i|f|o|2g.
      * c-path: sigma2c = Sigmoid(c, scale=2) (ACT's free affine).
      * h is kept at half scale: h' = (sigma2c - 1/2)*sigma_o = h/2; the 2x
        is folded into Whh and fc1_W host-side. All identities exact.
  - per stream per step: 2 ACT insts (down from 4), 4 DVE insts.
  - gates PSUM: per stream one [128,1024] region [i|f|o|2g], pool bufs=2 so
    next step's x-projection matmuls run during this step's elementwise.
ODE: 9 fixed-grid Kutta-3/8 steps, 4 odef evals each, 2 interleaved
  half-batch streams (OSTR=2):
  - relu+bias folded into one tensor_scalar op (in+b max 0) on DVE/GPSIMD,
    keeping ACT nearly free for the exp.
  - pn3 as one [3,OW] matmul (cols 1,2 negated host-side) + one [3,OW] Exp.
  - Rp+Rd^-1 broadcast via one K=3 selector matmul; C^-1 via another.

Engine instructions carry a single HW sync-wait slot; a post-Tile pass
moves excess waits onto same-engine NoOps.
"""

import sys
import numpy as np

for _p in ("/opt/trn_rl_repo",):
    if _p not in sys.path:
        sys.path.insert(0, _p)

import concourse.bass as bass
import concourse.tile as tile
import concourse.mybir as mybir
import concourse.bass_utils as _bu
from concourse.bass_utils import run_bass_kernel_spmd


def _patched_bir_verify_and_optimise(tmpdir, inp="bir.json", outp="file.neff",
                                     arch=None, *, dve_root=None):
    """Same as bass_utils.bir_verify_and_optimise but with walrus LDW
    dedup enabled (redundant LDWEIGHTS elision for back-to-back matmuls
    sharing a stationary operand)."""
    cmd = [
        _bu.get_walrus_driver(),
        "--pass",
        ",".join(["birverifier", "runtime_memory_reservation", "lower_act",
                  "lower_dve", "lower_ap_offset", "codegen", "neff_packager"]),
        "-i", inp,
        "--neff-output-filename", outp,
        "--enable-birsim=true", "--mem-mode=physical", "--policy=0",
        "--enable-ldw-opt=true",
        "--assign-static-dmas-to-sp=false",
        "--dram-page-size=256", "--enable-neff-debug-info=true",
        "--jobs", "8",
        *_bu.get_walrus_args(
            _bu.get_bir_arch(tmpdir, inp) if arch is None else arch,
            tmpdir, dve_root=dve_root),
    ]
    result = _bu.run_command(cmd, cwd=tmpdir)
    if result is not None:
        from pathlib import Path
        (Path(tmpdir) / "log.txt").write_text(result.stdout)
    return f"{tmpdir}/{outp}"


_bu.bir_verify_and_optimise = _patched_bir_verify_and_optimise

F32 = mybir.dt.float32
F32R = mybir.dt.float32r
AF = mybir.ActivationFunctionType
ALU = mybir.AluOpType

NCORES = 8
B, T_FULL, D_IN, H, LAT = 4096, 256, 2, 128, 128
BP = B // NCORES          # 512 batch per core
BS = BP // 2              # 256 per stream
N_STEPS = 9
SXT = 16                  # t-slots per xt3 tile (x rows 0..31, ones at 32)

# gate order in PSUM regions: i, f, o, g  (pytorch packs i, f, g, o)
GATE_PERM = (0, 1, 3, 2)

# weight tensors (fp32r tiles); bias tensors (f32 tiles)
_W_SPECS = [
    ("Wball", [128, SXT * 512]),
    ("Whh", [128, 512]),
    ("erows", [128, 384]),
    ("fc1W", [128, 256]),
    ("fc2W", [128, 256]),
    ("pn1W", [128, 128]),
    ("pn2W", [128, 128]),
    ("pn3W", [128, 3]),
    ("pn3Wpos", [128, 2]),
    ("cn1W", [128, 128]),
    ("cn2W", [128, 128]),
    ("cn3W", [128, 128]),
    ("dec1aW", [128, 128]),
    ("dec1b0W", [1, 128]), ("dec1b1W", [1, 128]), ("dec1b2W", [1, 128]),
    ("dec2W", [128, 128]),
    ("dec3W", [128, 2]),
]
_B_SPECS = [
    ("fc1b2", [128, 2]),
    ("fc2b", [128, 1]),
    ("pn1b", [128, 1]), ("pn2b", [128, 1]),
    ("pn3bias3", [3, 1]), ("pn3biasB", [1, 1]), ("pn3biasC", [1, 1]),
    ("cn1b", [128, 1]), ("cn2b", [128, 1]), ("cn3b", [128, 1]),
    ("dec1b", [128, 1]),
    ("dec2b", [128, 1]),
    ("dec3b", [2, 1]),
]


def _f32(ap):
    return ap.bitcast(F32)


def _legalize_matmul_waits(nc):
    """Engine instructions carry a single HW sync-wait slot (walrus: 'Too
    many sync wait commands'). Move excess waits onto preceding NoOps on the
    same engine queue; engine FIFO order keeps correctness."""
    n_moved = 0
    for fn in nc.m.functions:
        for bb in fn.blocks:
            out = []
            for inst in bb.instructions:
                si = inst.sync_info
                if si is not None and si.on_wait and len(si.on_wait) > 1:
                    waits = list(si.on_wait)
                    for w in waits[:-1]:
                        nop = mybir.InstNoOp(
                            name=nc.get_next_instruction_name(),
                            engine=inst.engine,
                            ins=[], outs=[],
                            sync_info=mybir.SyncInfo(on_wait=[w], on_update=[]),
                        )
                        out.append(nop)
                    si.on_wait = waits[-1:]
                    n_moved += 1
                out.append(inst)
            bb.instructions[:] = out
    return n_moved


def build_program(T=T_FULL, n_steps=N_STEPS, use_f32r=True, debug=False,
                  legalize=True):
    RD = F32R if use_f32r else F32
    dt = 1.0 / n_steps
    nxt = (T + SXT - 1) // SXT
    nc = bass.Bass()
    ins = {}
    ins["xt3"] = nc.declare_dram_parameter("xt3", [128, nxt * BP], RD,
                                           isOutput=False)
    # aux constants: cols 0:128 ones, 128:384 zeros (initial h)
    ins["aux"] = nc.declare_dram_parameter("aux", [128, 128 + 2 * BS], RD,
                                           isOutput=False)
    for name, shape in _W_SPECS:
        ins[name] = nc.declare_dram_parameter(name, shape, RD, isOutput=False)
    for name, shape in _B_SPECS:
        ins[name] = nc.declare_dram_parameter(name, shape, F32, isOutput=False)
    y_out = nc.declare_dram_parameter("y", [2, BP], F32, isOutput=True)
    if debug:
        dbg_h = nc.declare_dram_parameter("dbg_h", [128, BP], F32, isOutput=True)
        dbg_z0 = nc.declare_dram_parameter("dbg_z0", [128, BP], F32, isOutput=True)
        dbg_zT = nc.declare_dram_parameter("dbg_zT", [128, BP], F32, isOutput=True)
        dbg_pr = nc.declare_dram_parameter("dbg_pr", [3, BP], F32, isOutput=True)
        dbg_k = nc.declare_dram_parameter("dbg_k", [128, 4 * BP], F32,
                                          isOutput=True)

    with tile.TileContext(nc) as tc:
        with (
            tc.tile_pool(name="const", bufs=1) as cp,
            tc.tile_pool(name="state", bufs=2) as st,
        ):
            sb = {}
            sb["xt3"] = cp.tile([128, nxt * BP], RD, tag="xt3", name="xt3")
            nc.sync.dma_start(sb["xt3"][:], ins["xt3"][:])
            for name, shape in _W_SPECS:
                sb[name] = cp.tile(shape, RD, tag=name, name=name)
                nc.sync.dma_start(sb[name][:], ins[name][:])
            for name, shape in _B_SPECS:
                sb[name] = cp.tile(shape, F32, tag=name, name=name)
                nc.sync.dma_start(sb[name][:], ins[name][:])
            paramA = cp.tile([1, BP], RD, tag="paramA")
            paramB = cp.tile([1, BP], RD, tag="paramB")
            paramC = cp.tile([1, BP], RD, tag="paramC")

            h = []
            c = []
            for s in range(2):
                ht = st.tile([128, BS], RD, tag=f"h{s}")
                ct = st.tile([128, BS], F32, tag=f"c{s}")
                nc.sync.dma_start(
                    ht[:], ins["aux"][:, 128 + BS * s : 128 + BS * (s + 1)])
                nc.gpsimd.memset(ct[:], 0.0)
                h.append(ht)
                c.append(ct)

            xt3 = sb["xt3"]
            Wball = sb["Wball"]
            Whh = sb["Whh"]

            # ------------------ LSTM ------------------
            # per-stream gates psum [128,1024] = [i|f|o|2g], 256 cols each.
            # bufs=2: next step's x-projection matmuls (no h dependency) open
            # the other buffer's accumulation groups during this step's
            # elementwise chain.
            with (
                tc.tile_pool(name="psA", bufs=2, space="PSUM") as gp,
                tc.tile_pool(name="work", bufs=3) as wp,
            ):
                for t in range(T):
                    til, slot = divmod(t, SXT)
                    gates = {}
                    for s in range(2):
                        gates[s] = gp.tile([128, 1024], F32, tag=f"g{s}",
                                           name=f"g{s}_{t}")
                    # x-projection: 4 K=128 matmuls per stream (bias folded
                    # into Wball's ones row)
                    for s in range(2):
                        xsl = xt3[:, BP * til + BS * s
                                  : BP * til + BS * (s + 1)]
                        for ci in range(4):
                            nc.tensor.matmul(
                                gates[s][:, 256 * ci : 256 * (ci + 1)],
                                Wball[:, 512 * slot + 128 * ci
                                      : 512 * slot + 128 * (ci + 1)],
                                xsl,
                                start=True, stop=False)
                    # recurrent part
                    for s in range(2):
                        for ci in range(4):
                            nc.tensor.matmul(
                                gates[s][:, 256 * ci : 256 * (ci + 1)],
                                Whh[:, 128 * ci : 128 * (ci + 1)],
                                h[s][:],
                                start=False, stop=True)
                    # one sigmoid over [i|f|o|2g]; sigma(2g) = (tanh(g)+1)/2
                    sgm = {}
                    for s in range(2):
                        sgm[s] = wp.tile([128, 1024], F32, tag=f"sg{s}",
                                         name=f"sg{s}_{t}")
                        nc.scalar.activation(sgm[s][:], gates[s][:], AF.Sigmoid)
                    # c_new = 2*(sig2g - 1/2)*sig_i + sig_f*c
                    cn = {}
                    for s in range(2):
                        t1 = wp.tile([128, BS], F32, tag=f"t1{s}", name=f"t1{s}_{t}")
                        nc.vector.scalar_tensor_tensor(
                            out=t1[:], in0=sgm[s][:, 768:1024], scalar=0.5,
                            in1=sgm[s][:, 0:256], op0=ALU.subtract, op1=ALU.mult)
                        t2 = wp.tile([128, BS], F32, tag=f"t2{s}", name=f"t2{s}_{t}")
                        nc.vector.tensor_tensor(
                            out=t2[:], in0=sgm[s][:, 256:512], in1=c[s][:],
                            op=ALU.mult)
                        cn[s] = st.tile([128, BS], F32, tag=f"c{s}", name=f"c{s}_{t}")
                        nc.vector.scalar_tensor_tensor(
                            out=cn[s][:], in0=t1[:], scalar=2.0, in1=t2[:],
                            op0=ALU.mult, op1=ALU.add)
                        c[s] = cn[s]
                    # sigma(2c) on ACT (free scale), then h' = (sig2c-1/2)*sig_o
                    sgc = {}
                    for s in range(2):
                        sgc[s] = wp.tile([128, BS], F32, tag=f"tc{s}",
                                         name=f"tc{s}_{t}")
                        nc.scalar.activation(sgc[s][:], cn[s][:], AF.Sigmoid,
                                             scale=2.0)
                    for s in range(2):
                        hn_ = st.tile([128, BS], RD, tag=f"h{s}", name=f"h{s}_{t}")
                        nc.vector.scalar_tensor_tensor(
                            out=hn_[:], in0=sgc[s][:], scalar=0.5,
                            in1=sgm[s][:, 512:768], op0=ALU.subtract,
                            op1=ALU.mult)
                        h[s] = hn_

            # ------------- encoder fc + ODE + decoder -------------
            OSTR = 2
            OW = BP // OSTR
            with (
                tc.tile_pool(name="psB", bufs=1, space="PSUM") as pb,
                tc.tile_pool(name="ow", bufs=2) as ow,
            ):
                if debug:
                    for s in range(2):
                        nc.sync.dma_start(
                            dbg_h[:, BS * s : BS * (s + 1)], _f32(h[s][:]))
                # fc1: hN @ fc1W + b -> relu ; chunks j of the 256-dim output
                # (fc1W pre-doubled host-side: h is at half scale)
                r1 = ow.tile([128, 1024], RD, tag="r1")
                for j in range(2):
                    pfc = pb.tile([128, 512], F32, tag=f"ps1_{j}")
                    for s in range(2):
                        nc.tensor.matmul(
                            pfc[:, BS * s : BS * (s + 1)],
                            sb["fc1W"][:, 128 * j : 128 * (j + 1)],
                            h[s][:], start=True, stop=True)
                    nc.scalar.activation(
                        r1[:, 512 * j : 512 * (j + 1)], pfc[:], AF.Relu,
                        bias=sb["fc1b2"][:, j : j + 1])
                # fc2 (no relu)
                pz = pb.tile([128, BP], F32, tag="ps2_0")
                nc.tensor.matmul(pz[:], sb["fc2W"][:, 0:128], r1[:, 0:512],
                                 start=True, stop=False)
                nc.tensor.matmul(pz[:], sb["fc2W"][:, 128:256], r1[:, 512:1024],
                                 start=False, stop=True)
                zs = []
                for s_ in range(OSTR):
                    zt = ow.tile([128, OW], RD, tag=f"z{s_}")
                    nc.vector.tensor_scalar(
                        out=zt[:], in0=pz[:, OW * s_ : OW * (s_ + 1)],
                        scalar1=sb["fc2b"][:], scalar2=None, op0=ALU.add)
                    zs.append(zt)
                if debug:
                    for s_ in range(OSTR):
                        nc.sync.dma_start(dbg_z0[:, OW * s_ : OW * (s_ + 1)],
                                          _f32(zs[s_][:]))

                def relu_b(eng, out_ap, in_ap, bias_ap):
                    # out = max(in + b, 0) in one op on the given engine
                    eng.tensor_scalar(out=out_ap, in0=in_ap,
                                      scalar1=bias_ap, scalar2=0.0,
                                      op0=ALU.add, op1=ALU.max)

                def odef(zin, s_, first=False, ktag="k"):
                    sl = slice(OW * s_, OW * (s_ + 1))
                    # layer1: [pn1|cn1] into one psum bank, relu+bias on
                    # DVE (pn half) / ACT (cn half; GPSIMD can't read PSUM)
                    p1 = pb.tile([128, 512], F32, tag=f"ps1_{s_}", name="p1")
                    nc.tensor.matmul(p1[:, 0:256], sb["pn1W"][:], zin[:],
                                     start=True, stop=True)
                    nc.tensor.matmul(p1[:, 256:512], sb["cn1W"][:], zin[:],
                                     start=True, stop=True)
                    s1 = ow.tile([128, 512], RD, tag=f"s1_{s_}")
                    relu_b(nc.vector, s1[:, 0:256], p1[:, 0:256], sb["pn1b"][:])
                    nc.scalar.activation(s1[:, 256:512], p1[:, 256:512],
                                         AF.Relu, bias=sb["cn1b"][:])
                    # layer2
                    p2 = pb.tile([128, 512], F32, tag=f"ps2_{s_}", name="p2")
                    nc.tensor.matmul(p2[:, 0:256], sb["pn2W"][:], s1[:, 0:256],
                                     start=True, stop=True)
                    nc.tensor.matmul(p2[:, 256:512], sb["cn2W"][:],
                                     s1[:, 256:512], start=True, stop=True)
                    s2 = ow.tile([128, 512], RD, tag=f"s2_{s_}")
                    relu_b(nc.vector, s2[:, 0:256], p2[:, 0:256], sb["pn2b"][:])
                    nc.scalar.activation(s2[:, 256:512], p2[:, 256:512],
                                         AF.Relu, bias=sb["cn2b"][:])
                    # layer3: cn3 -> pcn cols 0:256; pn3 (cols 1,2 negated
                    # host-side) -> rows 0..2 of cols 256:512
                    p3 = pb.tile([128, 512], F32, tag=f"ps3_{s_}", name="p3")
                    nc.tensor.matmul(p3[:, 0:256], sb["cn3W"][:], s2[:, 256:512],
                                     start=True, stop=True)
                    nc.tensor.matmul(p3[0:3, 256:512], sb["pn3W"][:],
                                     s2[:, 0:256], start=True, stop=True)
                    # rows = [exp(p0+b0); exp(-p1-b1); exp(-p2-b2)] in one Exp
                    rows = ow.tile([3, OW], RD, tag=f"rows{s_}")
                    nc.scalar.activation(rows[:], p3[0:3, 256:512], AF.Exp,
                                         bias=sb["pn3bias3"][:], scale=1.0)
                    if first:
                        # params = exp(p + b) rows for the decoder; separate
                        # M=1 matmuls (psum base partition 0) + exp. Reuses
                        # the ps2 bank (s2's psum is already consumed).
                        pp = pb.tile([128, 512], F32, tag=f"ps2_{s_}",
                                     name="pp")
                        nc.tensor.matmul(pp[0:1, 0:256], sb["pn3Wpos"][:, 0:1],
                                         s2[:, 0:256], start=True, stop=True)
                        nc.tensor.matmul(pp[0:1, 256:512],
                                         sb["pn3Wpos"][:, 1:2],
                                         s2[:, 0:256], start=True, stop=True)
                        nc.vector.tensor_copy(out=paramA[0:1, sl],
                                              in_=rows[0:1, :])
                        nc.scalar.activation(paramB[0:1, sl], pp[0:1, 0:256],
                                             AF.Exp, bias=sb["pn3biasB"][:],
                                             scale=1.0)
                        nc.scalar.activation(paramC[0:1, sl], pp[0:1, 256:512],
                                             AF.Exp, bias=sb["pn3biasC"][:],
                                             scale=1.0)
                    # S_b = bcast(Rp + Rd^-1) via K=3 selector [1;1;0];
                    # C_b = bcast(C^-1) via K=3 selector [0;0;1]
                    p4 = pb.tile([128, 512], F32, tag=f"ps4_{s_}", name="p4")
                    nc.tensor.matmul(p4[:, 0:256], sb["erows"][0:3, 0:128],
                                     rows[:], start=True, stop=True)
                    nc.tensor.matmul(p4[:, 256:512], sb["erows"][0:3, 128:256],
                                     rows[:], start=True, stop=True)
                    # k = (comp + cn3b - z*S_b) * C_b
                    d1 = ow.tile([128, OW], F32, tag=f"d1{s_}")
                    nc.vector.tensor_tensor(out=d1[:], in0=_f32(zin[:]),
                                            in1=p4[:, 0:256], op=ALU.mult)
                    d2 = ow.tile([128, OW], F32, tag=f"d2{s_}")
                    nc.vector.scalar_tensor_tensor(
                        out=d2[:], in0=p3[:, 0:256], scalar=sb["cn3b"][:],
                        in1=d1[:], op0=ALU.add, op1=ALU.subtract)
                    k = ow.tile([128, OW], F32, tag=ktag)
                    nc.vector.tensor_tensor(out=k[:], in0=d2[:],
                                            in1=p4[:, 256:512], op=ALU.mult)
                    return k

                def sttz(k_in0, scalar, ztile, tag, eng=None):
                    # fp32r out: (k * scalar) + z
                    o = ow.tile([128, OW], RD, tag=tag)
                    (eng or nc.vector).scalar_tensor_tensor(
                        out=o[:], in0=k_in0[:], scalar=float(scalar),
                        in1=_f32(ztile[:]),
                        op0=ALU.mult, op1=ALU.add)
                    return o

                def sttk(in0, scalar, in1, tag, eng=None):
                    # f32 out: (in0 * scalar) + in1
                    o = ow.tile([128, OW], F32, tag=tag)
                    (eng or nc.vector).scalar_tensor_tensor(
                        out=o[:], in0=in0[:], scalar=float(scalar), in1=in1[:],
                        op0=ALU.mult, op1=ALU.add)
                    return o

                def tt(in0, in1, op, tag, eng=None):
                    o = ow.tile([128, OW], F32, tag=tag)
                    (eng or nc.vector).tensor_tensor(out=o[:], in0=in0[:],
                                                     in1=in1[:], op=op)
                    return o

                for step in range(n_steps):
                    for s_ in range(OSTR):
                        z = zs[s_]
                        k1 = odef(z, s_, first=(step == 0), ktag=f"k1{s_}")
                        za = sttz(k1, dt / 3.0, z, f"za{s_}")   # z + dt/3 k1
                        k2 = odef(za, s_, ktag=f"k2{s_}")
                        if debug and step == 0 and s_ == 0:
                            nc.sync.dma_start(dbg_k[:, 0:OW], k1[:])
                            nc.sync.dma_start(dbg_k[:, BP : BP + OW], k2[:])
                            nc.sync.dma_start(dbg_k[:, 2 * BP : 2 * BP + OW],
                                              _f32(za[:]))
                        u1 = sttk(k1, -1.0 / 3.0, k2, f"u1{s_}")  # k2 - k1/3
                        zb = sttz(u1, dt, z, f"za{s_}")  # z + dt(k2 - k1/3)
                        k3 = odef(zb, s_, ktag=f"k3{s_}")
                        u2 = tt(k1, k2, ALU.subtract, f"u1{s_}", eng=nc.gpsimd)
                        u3 = tt(u2, k3, ALU.add, f"u2{s_}", eng=nc.gpsimd)
                        zc2 = sttz(u3, dt, z, f"za{s_}")  # z + dt(k1 - k2 + k3)
                        k4 = odef(zc2, s_, ktag=f"k4{s_}")
                        v1 = tt(k2, k3, ALU.add, f"u1{s_}", eng=nc.gpsimd)
                        v2 = sttk(v1, 3.0, k1, f"u2{s_}")  # k1 + 3(k2 + k3)
                        v3 = tt(v2, k4, ALU.add, f"u1{s_}", eng=nc.gpsimd)
                        zs[s_] = sttz(v3, dt / 8.0, z, f"z{s_}")  # z + dt/8 (..)

                for s_ in range(OSTR):
                    sl = slice(OW * s_, OW * (s_ + 1))
                    if debug:
                        nc.sync.dma_start(dbg_zT[:, sl], _f32(zs[s_][:]))
                        if s_ == 0:
                            nc.sync.dma_start(dbg_pr[0:1, :], _f32(paramA[:]))
                            nc.sync.dma_start(dbg_pr[1:2, :], _f32(paramB[:]))
                            nc.sync.dma_start(dbg_pr[2:3, :], _f32(paramC[:]))
                    # decoder: zc = [zT ; params]
                    pd1 = pb.tile([128, 512], F32, tag=f"ps1_{s_}")
                    nc.tensor.matmul(pd1[:, 0:256], sb["dec1aW"][:], zs[s_][:],
                                     start=True, stop=False)
                    nc.tensor.matmul(pd1[:, 0:256], sb["dec1b0W"][:],
                                     paramA[0:1, sl], start=False, stop=False)
                    nc.tensor.matmul(pd1[:, 0:256], sb["dec1b1W"][:],
                                     paramB[0:1, sl], start=False, stop=False)
                    nc.tensor.matmul(pd1[:, 0:256], sb["dec1b2W"][:],
                                     paramC[0:1, sl], start=False, stop=True)
                    sd1 = ow.tile([128, OW], RD, tag=f"sd1{s_}")
                    nc.scalar.activation(sd1[:], pd1[:, 0:256], AF.Relu,
                                         bias=sb["dec1b"][:])
                    pd2 = pb.tile([128, 512], F32, tag=f"ps2_{s_}")
                    nc.tensor.matmul(pd2[:, 0:256], sb["dec2W"][:], sd1[:],
                                     start=True, stop=True)
                    sd2 = ow.tile([128, OW], RD, tag=f"sd2{s_}")
                    nc.scalar.activation(sd2[:], pd2[:, 0:256], AF.Relu,
                                         bias=sb["dec2b"][:])
                    pd3 = pb.tile([128, 512], F32, tag=f"ps3_{s_}")
                    nc.tensor.matmul(pd3[0:2, 0:256], sb["dec3W"][:], sd2[:],
                                     start=True, stop=True)
                    yt = ow.tile([2, OW], F32, tag=f"y{s_}")
                    nc.vector.tensor_scalar(out=yt[:], in0=pd3[0:2, 0:256],
                                            scalar1=sb["dec3b"][:],
                                            scalar2=None, op0=ALU.add)
                    nc.sync.dma_start(y_out[:, sl], yt[:])

    if legalize:
        _legalize_matmul_waits(nc)
    return nc


def prep_inputs(inputs, T=T_FULL):
    """Host-side marshaling: shard x, build xt3/Wball layouts, repack weights.

    Scaling conventions (exact identities, see module docstring):
      - h is stored at half scale -> Whh and fc1_W pre-multiplied by 2
      - g-gate preactivation doubled -> g columns of Wih/Whh/bias x2
      - pn3 columns 1,2 negated so one Exp produces [Rp, Rd^-1, C^-1]
    """
    nxt = (T + SXT - 1) // SXT
    f = lambda a: np.ascontiguousarray(a, dtype=np.float32)
    x = f(inputs["x"])                      # [B, T, 2]
    Wih = f(inputs["lstm_Wih"])             # [2, 512]
    Whh = f(inputs["lstm_Whh"])             # [128, 512]
    bsum = f(inputs["lstm_bih"] + inputs["lstm_bhh"])   # [512]

    # permute gate chunks (i, f, g, o) -> (i, f, o, g)
    def permc(w):
        chunks = [w[..., 128 * cc : 128 * (cc + 1)] for cc in GATE_PERM]
        return np.concatenate(chunks, axis=-1)

    Wih_p, Whh_p, bsum_p = permc(Wih), permc(Whh), permc(bsum)

    # double the g-gate preactivation (tanh -> sigmoid trick)
    Wih_p = Wih_p.copy(); Whh_p = Whh_p.copy(); bsum_p = bsum_p.copy()
    Wih_p[:, 384:512] *= 2.0
    bsum_p[384:512] *= 2.0
    Whh_p[:, 384:512] *= 2.0
    # h stored at half scale
    Whh_p *= 2.0

    # Wball: [128, SXT*512]; slot s: rows 2s,2s+1 = Wih rows, row 32 = bias
    Wball = np.zeros((128, SXT * 512), dtype=np.float32)
    for s in range(SXT):
        Wball[2 * s, 512 * s : 512 * (s + 1)] = Wih_p[0]
        Wball[2 * s + 1, 512 * s : 512 * (s + 1)] = Wih_p[1]
        Wball[32, 512 * s : 512 * (s + 1)] = bsum_p

    # xt3 per core: [128, nxt*BP]; tile t//SXT, x rows 2(t%SXT), ones row 32
    xt3_all = np.zeros((NCORES, 128, nxt * BP), dtype=np.float32)
    xs = x.reshape(NCORES, BP, T, 2)
    for core in range(NCORES):
        xc = xs[core]                       # [BP, T, 2]
        for t in range(T):
            til, slot = divmod(t, SXT)
            col0 = BP * til
            xt3_all[core, 2 * slot, col0 : col0 + BP] = xc[:, t, 0]
            xt3_all[core, 2 * slot + 1, col0 : col0 + BP] = xc[:, t, 1]
        xt3_all[core, 32, :] = 1.0

    # selector rows for the ODE broadcasts: cols 0:128 -> rows0+rows1 (S_b),
    # cols 128:256 -> rows2 (C_b)
    erows = np.zeros((128, 384), dtype=np.float32)
    erows[0, 0:128] = 1.0
    erows[1, 0:128] = 1.0
    erows[2, 128:256] = 1.0

    def padw(w, rows, cols):
        out = np.zeros((rows, cols), dtype=np.float32)
        out[: w.shape[0], : w.shape[1]] = w
        return out

    def padb(b, rows):
        out = np.zeros((rows, 1), dtype=np.float32)
        out[: b.shape[0], 0] = b
        return out

    fc1_b = f(inputs["fc1_b"])
    fc2_W = f(inputs["fc2_W"])
    pn3_W = f(inputs["pn3_W"])              # [128, 3]
    pn3_b = f(inputs["pn3_b"])
    # negate cols 1,2 so exp([p0, -p1, -p2] + [b0, -b1, -b2]) gives
    # [Rp, Rd^-1, C^-1] in one activation
    pn3W_mod = pn3_W * np.array([1.0, -1.0, -1.0], dtype=np.float32)
    pn3bias3 = np.array([[pn3_b[0]], [-pn3_b[1]], [-pn3_b[2]]],
                        dtype=np.float32)
    dec1_W = f(inputs["dec1_W"])            # [131, 128]

    common = {
        "Wball": Wball,
        "Whh": f(Whh_p),
        "erows": erows,
        "fc1W": f(inputs["fc1_W"] * 2.0),
        "fc1b2": f(fc1_b.reshape(2, 128).T),
        "fc2W": f(np.concatenate([fc2_W[0:128], fc2_W[128:256]], axis=1)),
        "fc2b": f(inputs["fc2_b"][:, None]),
        "pn1W": padw(f(inputs["pn1_W"]), 128, 128),
        "pn1b": padb(f(inputs["pn1_b"]), 128),
        "pn2W": padw(f(inputs["pn2_W"]), 128, 128),
        "pn2b": f(inputs["pn2_b"][:, None]),
        "pn3W": f(pn3W_mod),
        "pn3Wpos": f(pn3_W[:, 1:3]),
        "pn3bias3": pn3bias3,
        "pn3biasB": np.array([[pn3_b[1]]], dtype=np.float32),
        "pn3biasC": np.array([[pn3_b[2]]], dtype=np.float32),
        "cn1W": padw(f(inputs["cn1_W"]), 128, 128),
        "cn1b": padb(f(inputs["cn1_b"]), 128),
        "cn2W": padw(f(inputs["cn2_W"]), 128, 128),
        "cn2b": f(inputs["cn2_b"][:, None]),
        "cn3W": f(inputs["cn3_W"]), "cn3b": f(inputs["cn3_b"][:, None]),
        "dec1aW": f(dec1_W[0:128]),
        "dec1b0W": f(dec1_W[128:129]), "dec1b1W": f(dec1_W[129:130]),
        "dec1b2W": f(dec1_W[130:131]),
        "dec1b": f(inputs["dec1_b"][:, None]),
        "dec2W": padw(f(inputs["dec2_W"]), 128, 128),
        "dec2b": padb(f(inputs["dec2_b"]), 128),
        "dec3W": padw(f(inputs["dec3_W"]), 128, 2),
        "dec3b": f(inputs["dec3_b"][:, None]),
    }
    aux = np.zeros((128, 128 + 2 * BS), dtype=np.float32)
    aux[:, 0:128] = 1.0
    common["aux"] = aux

    in_maps = []
    for core in range(NCORES):
        m = dict(common)
        m["xt3"] = xt3_all[core]
        in_maps.append(m)
    return in_maps


_PROGRAM = None


def get_program():
    global _PROGRAM
    if _PROGRAM is None:
        _PROGRAM = build_program()
    return _PROGRAM


def run(inputs, **kwargs):
    nc = get_program()
    in_maps = prep_inputs(inputs)
    res = run_bass_kernel_spmd(nc, in_maps, list(range(NCORES)), **kwargs)
    outs = [res.results[i]["y"] for i in range(NCORES)]   # each [2, BP]
    y = np.concatenate([o.T for o in outs], axis=0).astype(np.float32)  # [B, 2]
    return y, res


def kernel(**inputs):
    y, _ = run(inputs)
    return y


# revision 16
# speedup vs baseline: 1.0353x; 1.0353x over previous
"""Trainium2 Bass kernel for nn_BPModel: LSTM encoder -> latent ODE (RK4) -> decoder.

Data-parallel over 8 NeuronCores: batch 4096 -> 512 per core. All parameters
replicated. Everything stays on-chip (SBUF) in a transposed [feature, batch]
layout; matmuls run as fp32r (1 col/cycle at the PE when N>=256).

v2 redesign (vs v1 baseline at 1.85ms): the ACT (scalar) engine was the
bottleneck at 77% busy in the LSTM phase (8 sigmoid/tanh insts per step).
Key changes:

LSTM (T=256 steps, 2 interleaved half-batch streams of 256 cols each):
  - tanh eliminated via tanh(x) = 2*sigmoid(2x) - 1:
      * g-gate preactivation is pre-doubled (host scales the g columns of
        Wih/Whh/bias by 2), so one [128,1024] Sigmoid covers i|f|o|2g.
      * c-path: sigma2c = Sigmoid(c, scale=2) (ACT's free affine).
      * h is kept at half scale: h' = (sigma2c - 1/2)*sigma_o = h/2; the 2x
        is folded into Whh and fc1_W host-side. All identities exact.
  - per stream per step: 2 ACT insts (down from 4), 4 DVE insts.
  - gates PSUM: per stream one [128,1024] region [i|f|o|2g], pool bufs=2 so
    next step's x-projection matmuls run during this step's elementwise.
ODE: 9 fixed-grid Kutta-3/8 steps, 4 odef evals each, 2 interleaved
  half-batch streams (OSTR=2):
  - relu+bias folded into one tensor_scalar op (in+b max 0) on DVE/GPSIMD,
    keeping ACT nearly free for the exp.
  - pn3 as one [3,OW] matmul (cols 1,2 negated host-side) + one [3,OW] Exp.
  - Rp+Rd^-1 broadcast via one K=3 selector matmul; C^-1 via another.

Engine instructions carry a single HW sync-wait slot; a post-Tile pass
moves excess waits onto same-engine NoOps.
"""

import sys
import numpy as np

for _p in ("/opt/trn_rl_repo",):
    if _p not in sys.path:
        sys.path.insert(0, _p)

import concourse.bass as bass
import concourse.tile as tile
import concourse.mybir as mybir
import concourse.bass_utils as _bu
from concourse.bass_utils import run_bass_kernel_spmd


def _patched_bir_verify_and_optimise(tmpdir, inp="bir.json", outp="file.neff",
                                     arch=None, *, dve_root=None):
    """Same as bass_utils.bir_verify_and_optimise but with walrus LDW
    dedup enabled (redundant LDWEIGHTS elision for back-to-back matmuls
    sharing a stationary operand)."""
    cmd = [
        _bu.get_walrus_driver(),
        "--pass",
        ",".join(["birverifier", "runtime_memory_reservation", "lower_act",
                  "lower_dve", "lower_ap_offset", "codegen", "neff_packager"]),
        "-i", inp,
        "--neff-output-filename", outp,
        "--enable-birsim=true", "--mem-mode=physical", "--policy=0",
        "--enable-ldw-opt=true",
        "--assign-static-dmas-to-sp=false",
        "--dram-page-size=256", "--enable-neff-debug-info=true",
        "--jobs", "8",
        *_bu.get_walrus_args(
            _bu.get_bir_arch(tmpdir, inp) if arch is None else arch,
            tmpdir, dve_root=dve_root),
    ]
    result = _bu.run_command(cmd, cwd=tmpdir)
    if result is not None:
        from pathlib import Path
        (Path(tmpdir) / "log.txt").write_text(result.stdout)
    return f"{tmpdir}/{outp}"


_bu.bir_verify_and_optimise = _patched_bir_verify_and_optimise

F32 = mybir.dt.float32
F32R = mybir.dt.float32r
BF16 = mybir.dt.bfloat16
AF = mybir.ActivationFunctionType
ALU = mybir.AluOpType

NCORES = 8
B, T_FULL, D_IN, H, LAT = 4096, 256, 2, 128, 128
BP = B // NCORES          # 512 batch per core
BS = BP // 2              # 256 per stream
N_STEPS = 9
SXT = 16                  # t-slots per xt3 tile (x rows 0..31, ones at 32)

# gate order in PSUM regions: i, f, o, g  (pytorch packs i, f, g, o)
GATE_PERM = (0, 1, 3, 2)

# weight tensors (fp32r tiles); bias tensors (f32 tiles)
_W_SPECS = [
    ("Wball", [128, SXT * 512]),
    ("Whh", [128, 512]),
    ("erows", [128, 384]),
    ("fc1W", [128, 256]),
    ("fc2W", [128, 256]),
    ("pn1W", [128, 128]),
    ("pn2W", [128, 128]),
    ("pn3W", [128, 3]),
    ("pn3Wpos", [128, 2]),
    ("cn1W", [128, 128]),
    ("cn2W", [128, 128]),
    ("cn3W", [128, 128]),
    ("dec1aW", [128, 128]),
    ("dec1b0W", [1, 128]), ("dec1b1W", [1, 128]), ("dec1b2W", [1, 128]),
    ("dec2W", [128, 128]),
    ("dec3W", [128, 2]),
]
_B_SPECS = [
    ("fc1b2", [128, 2]),
    ("fc2b", [128, 1]),
    ("pn1b", [128, 1]), ("pn2b", [128, 1]),
    ("pn3bias3", [3, 1]), ("pn3biasB", [1, 1]), ("pn3biasC", [1, 1]),
    ("cn1b", [128, 1]), ("cn2b", [128, 1]), ("cn3b", [128, 1]),
    ("dec1b", [128, 1]),
    ("dec2b", [128, 1]),
    ("dec3b", [2, 1]),
]


def _f32(ap):
    return ap.bitcast(F32)


def _legalize_matmul_waits(nc):
    """Engine instructions carry a single HW sync-wait slot (walrus: 'Too
    many sync wait commands'). Move excess waits onto preceding NoOps on the
    same engine queue; engine FIFO order keeps correctness."""
    n_moved = 0
    for fn in nc.m.functions:
        for bb in fn.blocks:
            out = []
            for inst in bb.instructions:
                si = inst.sync_info
                if si is not None and si.on_wait and len(si.on_wait) > 1:
                    waits = list(si.on_wait)
                    for w in waits[:-1]:
                        nop = mybir.InstNoOp(
                            name=nc.get_next_instruction_name(),
                            engine=inst.engine,
                            ins=[], outs=[],
                            sync_info=mybir.SyncInfo(on_wait=[w], on_update=[]),
                        )
                        out.append(nop)
                    si.on_wait = waits[-1:]
                    n_moved += 1
                out.append(inst)
            bb.instructions[:] = out
    return n_moved


def build_program(T=T_FULL, n_steps=N_STEPS, use_f32r=True, debug=False,
                  legalize=True):
    RD = F32R if use_f32r else F32
    dt = 1.0 / n_steps
    nxt = (T + SXT - 1) // SXT
    nc = bass.Bass()
    ins = {}
    ins["xt3"] = nc.declare_dram_parameter("xt3", [128, nxt * BP], RD,
                                           isOutput=False)
    # aux constants: cols 0:128 ones, 128:384 zeros (initial h)
    ins["aux"] = nc.declare_dram_parameter("aux", [128, 128 + 2 * BS], RD,
                                           isOutput=False)
    for name, shape in _W_SPECS:
        ins[name] = nc.declare_dram_parameter(name, shape, RD, isOutput=False)
    for name, shape in _B_SPECS:
        ins[name] = nc.declare_dram_parameter(name, shape, F32, isOutput=False)
    y_out = nc.declare_dram_parameter("y", [2, BP], F32, isOutput=True)
    if debug:
        dbg_h = nc.declare_dram_parameter("dbg_h", [128, BP], F32, isOutput=True)
        dbg_z0 = nc.declare_dram_parameter("dbg_z0", [128, BP], F32, isOutput=True)
        dbg_zT = nc.declare_dram_parameter("dbg_zT", [128, BP], F32, isOutput=True)
        dbg_pr = nc.declare_dram_parameter("dbg_pr", [3, BP], F32, isOutput=True)
        dbg_k = nc.declare_dram_parameter("dbg_k", [128, 4 * BP], F32,
                                          isOutput=True)

    with tile.TileContext(nc) as tc:
        with (
            tc.tile_pool(name="const", bufs=1) as cp,
            tc.tile_pool(name="state", bufs=2) as st,
        ):
            sb = {}
            sb["xt3"] = cp.tile([128, nxt * BP], RD, tag="xt3", name="xt3")
            # split the 4MB xt3 DMA so step 0's x-projection only waits for
            # the first time-tile
            nc.sync.dma_start(sb["xt3"][:, 0:BP], ins["xt3"][:, 0:BP])
            nc.sync.dma_start(sb["xt3"][:, BP:], ins["xt3"][:, BP:])
            for name, shape in _W_SPECS:
                sb[name] = cp.tile(shape, RD, tag=name, name=name)
                nc.sync.dma_start(sb[name][:], ins[name][:])
            for name, shape in _B_SPECS:
                sb[name] = cp.tile(shape, F32, tag=name, name=name)
                nc.sync.dma_start(sb[name][:], ins[name][:])
            paramA = cp.tile([1, BP], RD, tag="paramA")
            paramB = cp.tile([1, BP], RD, tag="paramB")
            paramC = cp.tile([1, BP], RD, tag="paramC")

            h = []
            c = []
            for s in range(2):
                ht = st.tile([128, BS], RD, tag=f"h{s}")
                ct = st.tile([128, BS], BF16, tag=f"c{s}")
                nc.sync.dma_start(
                    ht[:], ins["aux"][:, 128 + BS * s : 128 + BS * (s + 1)])
                nc.gpsimd.memset(ct[:], 0.0)
                h.append(ht)
                c.append(ct)

            xt3 = sb["xt3"]
            Wball = sb["Wball"]
            Whh = sb["Whh"]

            # ------------------ LSTM ------------------
            # per-stream gates psum [128,1024] = [i|f|o|2g], 256 cols each.
            # bufs=2: next step's x-projection matmuls (no h dependency) open
            # the other buffer's accumulation groups during this step's
            # elementwise chain.
            with (
                tc.tile_pool(name="psA", bufs=2, space="PSUM") as gp,
                tc.tile_pool(name="work", bufs=3) as wp,
            ):
                for t in range(T):
                    til, slot = divmod(t, SXT)
                    gates = {}
                    for s in range(2):
                        gates[s] = gp.tile([128, 1024], F32, tag=f"g{s}",
                                           name=f"g{s}_{t}")
                    # x-projection: 4 K=128 matmuls per stream (bias folded
                    # into Wball's ones row); ci-outer so consecutive matmuls
                    # share the stationary operand
                    for ci in range(4):
                        for s in range(2):
                            xsl = xt3[:, BP * til + BS * s
                                      : BP * til + BS * (s + 1)]
                            nc.tensor.matmul(
                                gates[s][:, 256 * ci : 256 * (ci + 1)],
                                Wball[:, 512 * slot + 128 * ci
                                      : 512 * slot + 128 * (ci + 1)],
                                xsl,
                                start=True, stop=False)
                    # recurrent part
                    for s in range(2):
                        for ci in range(4):
                            nc.tensor.matmul(
                                gates[s][:, 256 * ci : 256 * (ci + 1)],
                                Whh[:, 128 * ci : 128 * (ci + 1)],
                                h[s][:],
                                start=False, stop=True)
                    # one sigmoid over [i|f|o|2g]; sigma(2g) = (tanh(g)+1)/2
                    # bf16 outputs: DVE 2x mode on the elementwise chain
                    sgm = {}
                    for s in range(2):
                        sgm[s] = wp.tile([128, 1024], BF16, tag=f"sg{s}",
                                         name=f"sg{s}_{t}")
                        nc.scalar.activation(sgm[s][:], gates[s][:], AF.Sigmoid)
                    # c_new = 2*(sig2g - 1/2)*sig_i + sig_f*c
                    cn = {}
                    for s in range(2):
                        t1 = wp.tile([128, BS], BF16, tag=f"t1{s}", name=f"t1{s}_{t}")
                        nc.vector.scalar_tensor_tensor(
                            out=t1[:], in0=sgm[s][:, 768:1024], scalar=0.5,
                            in1=sgm[s][:, 0:256], op0=ALU.subtract, op1=ALU.mult)
                        t2 = wp.tile([128, BS], BF16, tag=f"t2{s}", name=f"t2{s}_{t}")
                        nc.vector.tensor_tensor(
                            out=t2[:], in0=sgm[s][:, 256:512], in1=c[s][:],
                            op=ALU.mult)
                        cn[s] = st.tile([128, BS], BF16, tag=f"c{s}", name=f"c{s}_{t}")
                        nc.vector.scalar_tensor_tensor(
                            out=cn[s][:], in0=t1[:], scalar=2.0, in1=t2[:],
                            op0=ALU.mult, op1=ALU.add)
                        c[s] = cn[s]
                    # sigma(2c) on ACT (free scale), then h' = (sig2c-1/2)*sig_o
                    sgc = {}
                    for s in range(2):
                        sgc[s] = wp.tile([128, BS], BF16, tag=f"tc{s}",
                                         name=f"tc{s}_{t}")
                        nc.scalar.activation(sgc[s][:], cn[s][:], AF.Sigmoid,
                                             scale=2.0)
                    for s in range(2):
                        hn_ = st.tile([128, BS], RD, tag=f"h{s}", name=f"h{s}_{t}")
                        nc.vector.scalar_tensor_tensor(
                            out=hn_[:], in0=sgc[s][:], scalar=0.5,
                            in1=sgm[s][:, 512:768], op0=ALU.subtract,
                            op1=ALU.mult)
                        h[s] = hn_

            # ------------- encoder fc + ODE + decoder -------------
            OSTR = 2
            OW = BP // OSTR
            with (
                tc.tile_pool(name="psB", bufs=1, space="PSUM") as pb,
                tc.tile_pool(name="ow", bufs=2) as ow,
            ):
                if debug:
                    for s in range(2):
                        nc.sync.dma_start(
                            dbg_h[:, BS * s : BS * (s + 1)], _f32(h[s][:]))
                # fc1: hN @ fc1W + b -> relu ; chunks j of the 256-dim output
                # (fc1W pre-doubled host-side: h is at half scale)
                r1 = ow.tile([128, 1024], RD, tag="r1")
                for j in range(2):
                    pfc = pb.tile([128, 512], F32, tag=f"ps1_{j}")
                    for s in range(2):
                        nc.tensor.matmul(
                            pfc[:, BS * s : BS * (s + 1)],
                            sb["fc1W"][:, 128 * j : 128 * (j + 1)],
                            h[s][:], start=True, stop=True)
                    nc.scalar.activation(
                        r1[:, 512 * j : 512 * (j + 1)], pfc[:], AF.Relu,
                        bias=sb["fc1b2"][:, j : j + 1])
                # fc2 (no relu)
                pz = pb.tile([128, BP], F32, tag="ps2_0")
                nc.tensor.matmul(pz[:], sb["fc2W"][:, 0:128], r1[:, 0:512],
                                 start=True, stop=False)
                nc.tensor.matmul(pz[:], sb["fc2W"][:, 128:256], r1[:, 512:1024],
                                 start=False, stop=True)
                zs = []
                for s_ in range(OSTR):
                    zt = ow.tile([128, OW], RD, tag=f"z{s_}")
                    nc.vector.tensor_scalar(
                        out=zt[:], in0=pz[:, OW * s_ : OW * (s_ + 1)],
                        scalar1=sb["fc2b"][:], scalar2=None, op0=ALU.add)
                    zs.append(zt)
                if debug:
                    for s_ in range(OSTR):
                        nc.sync.dma_start(dbg_z0[:, OW * s_ : OW * (s_ + 1)],
                                          _f32(zs[s_][:]))

                def relu_b(eng, out_ap, in_ap, bias_ap):
                    # out = max(in + b, 0) in one op on the given engine
                    eng.tensor_scalar(out=out_ap, in0=in_ap,
                                      scalar1=bias_ap, scalar2=0.0,
                                      op0=ALU.add, op1=ALU.max)

                def odef2(zins, first=False, ktag="k"):
                    """One odef eval for BOTH streams, emitted stage-
                    interleaved so the two chains pipeline on the in-order
                    engine FIFOs."""
                    p1 = {}; s1 = {}; p2 = {}; s2 = {}; p3 = {}; p4 = {}
                    rows = {}; d1 = {}; d2 = {}; k = {}
                    for s_ in range(OSTR):
                        # layer1: [pn1|cn1] into one psum bank
                        p1[s_] = pb.tile([128, 512], F32, tag=f"ps1_{s_}",
                                         name=f"p1_{s_}")
                        nc.tensor.matmul(p1[s_][:, 0:256], sb["pn1W"][:],
                                         zins[s_][:], start=True, stop=True)
                        nc.tensor.matmul(p1[s_][:, 256:512], sb["cn1W"][:],
                                         zins[s_][:], start=True, stop=True)
                    for s_ in range(OSTR):
                        # relu+bias: DVE (pn half) / ACT (cn half)
                        s1[s_] = ow.tile([128, 512], RD, tag=f"s1_{s_}", name=f"s1_{s_}")
                        relu_b(nc.vector, s1[s_][:, 0:256], p1[s_][:, 0:256],
                               sb["pn1b"][:])
                        nc.scalar.activation(s1[s_][:, 256:512],
                                             p1[s_][:, 256:512],
                                             AF.Relu, bias=sb["cn1b"][:])
                    for s_ in range(OSTR):
                        p2[s_] = pb.tile([128, 512], F32, tag=f"ps2_{s_}",
                                         name=f"p2_{s_}")
                        nc.tensor.matmul(p2[s_][:, 0:256], sb["pn2W"][:],
                                         s1[s_][:, 0:256], start=True, stop=True)
                        nc.tensor.matmul(p2[s_][:, 256:512], sb["cn2W"][:],
                                         s1[s_][:, 256:512], start=True,
                                         stop=True)
                    for s_ in range(OSTR):
                        s2[s_] = ow.tile([128, 512], RD, tag=f"s2_{s_}", name=f"s2_{s_}")
                        relu_b(nc.vector, s2[s_][:, 0:256], p2[s_][:, 0:256],
                               sb["pn2b"][:])
                        nc.scalar.activation(s2[s_][:, 256:512],
                                             p2[s_][:, 256:512],
                                             AF.Relu, bias=sb["cn2b"][:])
                    for s_ in range(OSTR):
                        # layer3: cn3 -> cols 0:256; pn3 (cols 1,2 negated
                        # host-side) -> rows 0..2 of cols 256:512
                        p3[s_] = pb.tile([128, 512], F32, tag=f"ps3_{s_}",
                                         name=f"p3_{s_}")
                        nc.tensor.matmul(p3[s_][:, 0:256], sb["cn3W"][:],
                                         s2[s_][:, 256:512], start=True,
                                         stop=True)
                        nc.tensor.matmul(p3[s_][0:3, 256:512], sb["pn3W"][:],
                                         s2[s_][:, 0:256], start=True,
                                         stop=True)
                    for s_ in range(OSTR):
                        # rows = [exp(p0+b0); exp(-p1-b1); exp(-p2-b2)]
                        rows[s_] = ow.tile([3, OW], RD, tag=f"rows{s_}", name=f"rows{s_}")
                        nc.scalar.activation(rows[s_][:], p3[s_][0:3, 256:512],
                                             AF.Exp, bias=sb["pn3bias3"][:],
                                             scale=1.0)
                    if first:
                        for s_ in range(OSTR):
                            sl = slice(OW * s_, OW * (s_ + 1))
                            # params = exp(p + b) rows for the decoder;
                            # M=1 matmuls (psum base partition 0) + exp.
                            pp = pb.tile([128, 512], F32, tag=f"ps2_{s_}",
                                         name=f"pp{s_}")
                            nc.tensor.matmul(pp[0:1, 0:256],
                                             sb["pn3Wpos"][:, 0:1],
                                             s2[s_][:, 0:256], start=True,
                                             stop=True)
                            nc.tensor.matmul(pp[0:1, 256:512],
                                             sb["pn3Wpos"][:, 1:2],
                                             s2[s_][:, 0:256], start=True,
                                             stop=True)
                            nc.vector.tensor_copy(out=paramA[0:1, sl],
                                                  in_=rows[s_][0:1, :])
                            nc.scalar.activation(paramB[0:1, sl],
                                                 pp[0:1, 0:256],
                                                 AF.Exp, bias=sb["pn3biasB"][:],
                                                 scale=1.0)
                            nc.scalar.activation(paramC[0:1, sl],
                                                 pp[0:1, 256:512],
                                                 AF.Exp, bias=sb["pn3biasC"][:],
                                                 scale=1.0)
                    for s_ in range(OSTR):
                        # S_b = bcast(Rp + Rd^-1) via K=3 selector [1;1;0];
                        # C_b = bcast(C^-1) via K=3 selector [0;0;1]
                        p4[s_] = pb.tile([128, 512], F32, tag=f"ps4_{s_}",
                                         name=f"p4_{s_}")
                        nc.tensor.matmul(p4[s_][:, 0:256],
                                         sb["erows"][0:3, 0:128],
                                         rows[s_][:], start=True, stop=True)
                        nc.tensor.matmul(p4[s_][:, 256:512],
                                         sb["erows"][0:3, 128:256],
                                         rows[s_][:], start=True, stop=True)
                    for s_ in range(OSTR):
                        # k = (comp + cn3b - z*S_b) * C_b
                        d1[s_] = ow.tile([128, OW], F32, tag=f"d1{s_}", name=f"d1{s_}")
                        nc.vector.tensor_tensor(out=d1[s_][:],
                                                in0=_f32(zins[s_][:]),
                                                in1=p4[s_][:, 0:256],
                                                op=ALU.mult)
                    for s_ in range(OSTR):
                        d2[s_] = ow.tile([128, OW], F32, tag=f"d2{s_}", name=f"d2{s_}")
                        nc.vector.scalar_tensor_tensor(
                            out=d2[s_][:], in0=p3[s_][:, 0:256],
                            scalar=sb["cn3b"][:],
                            in1=d1[s_][:], op0=ALU.add, op1=ALU.subtract)
                    for s_ in range(OSTR):
                        k[s_] = ow.tile([128, OW], F32, tag=f"{ktag}{s_}", name=f"{ktag}{s_}")
                        nc.vector.tensor_tensor(out=k[s_][:], in0=d2[s_][:],
                                                in1=p4[s_][:, 256:512],
                                                op=ALU.mult)
                    return k

                def sttz2(ks, scalar, ztiles, tag):
                    # fp32r out per stream: (k * scalar) + z
                    o = {}
                    for s_ in range(OSTR):
                        o[s_] = ow.tile([128, OW], RD, tag=f"{tag}{s_}", name=f"{tag}{s_}")
                        nc.vector.scalar_tensor_tensor(
                            out=o[s_][:], in0=ks[s_][:], scalar=float(scalar),
                            in1=_f32(ztiles[s_][:]),
                            op0=ALU.mult, op1=ALU.add)
                    return o

                def sttk2(ins0, scalar, ins1, tag):
                    # f32 out per stream: (in0 * scalar) + in1
                    o = {}
                    for s_ in range(OSTR):
                        o[s_] = ow.tile([128, OW], F32, tag=f"{tag}{s_}", name=f"{tag}{s_}")
                        nc.vector.scalar_tensor_tensor(
                            out=o[s_][:], in0=ins0[s_][:], scalar=float(scalar),
                            in1=ins1[s_][:], op0=ALU.mult, op1=ALU.add)
                    return o

                def tt2(ins0, ins1, op, tag, eng=None):
                    o = {}
                    for s_ in range(OSTR):
                        o[s_] = ow.tile([128, OW], F32, tag=f"{tag}{s_}", name=f"{tag}{s_}")
                        (eng or nc.vector).tensor_tensor(
                            out=o[s_][:], in0=ins0[s_][:], in1=ins1[s_][:],
                            op=op)
                    return o

                zd = {s_: zs[s_] for s_ in range(OSTR)}
                for step in range(n_steps):
                    z = dict(zd)
                    k1 = odef2(z, first=(step == 0), ktag="k1")
                    za = sttz2(k1, dt / 3.0, z, "za")       # z + dt/3 k1
                    k2 = odef2(za, ktag="k2")
                    u1 = sttk2(k1, -1.0 / 3.0, k2, "u1")    # k2 - k1/3
                    zb = sttz2(u1, dt, z, "za")             # z + dt(k2 - k1/3)
                    k3 = odef2(zb, ktag="k3")
                    u2 = tt2(k1, k2, ALU.subtract, "u1", eng=nc.gpsimd)
                    u3 = tt2(u2, k3, ALU.add, "u2", eng=nc.gpsimd)
                    zc2 = sttz2(u3, dt, z, "za")            # z + dt(k1-k2+k3)
                    k4 = odef2(zc2, ktag="k4")
                    v1 = tt2(k2, k3, ALU.add, "u1", eng=nc.gpsimd)
                    v2 = sttk2(v1, 3.0, k1, "u2")           # k1 + 3(k2 + k3)
                    v3 = tt2(v2, k4, ALU.add, "u1", eng=nc.gpsimd)
                    zd = sttz2(v3, dt / 8.0, z, "z")        # z + dt/8 (..)
                zs = [zd[s_] for s_ in range(OSTR)]

                for s_ in range(OSTR):
                    sl = slice(OW * s_, OW * (s_ + 1))
                    if debug:
                        nc.sync.dma_start(dbg_zT[:, sl], _f32(zs[s_][:]))
                        if s_ == 0:
                            nc.sync.dma_start(dbg_pr[0:1, :], _f32(paramA[:]))
                            nc.sync.dma_start(dbg_pr[1:2, :], _f32(paramB[:]))
                            nc.sync.dma_start(dbg_pr[2:3, :], _f32(paramC[:]))
                    # decoder: zc = [zT ; params]
                    pd1 = pb.tile([128, 512], F32, tag=f"ps1_{s_}")
                    nc.tensor.matmul(pd1[:, 0:256], sb["dec1aW"][:], zs[s_][:],
                                     start=True, stop=False)
                    nc.tensor.matmul(pd1[:, 0:256], sb["dec1b0W"][:],
                                     paramA[0:1, sl], start=False, stop=False)
                    nc.tensor.matmul(pd1[:, 0:256], sb["dec1b1W"][:],
                                     paramB[0:1, sl], start=False, stop=False)
                    nc.tensor.matmul(pd1[:, 0:256], sb["dec1b2W"][:],
                                     paramC[0:1, sl], start=False, stop=True)
                    sd1 = ow.tile([128, OW], RD, tag=f"sd1{s_}")
                    nc.scalar.activation(sd1[:], pd1[:, 0:256], AF.Relu,
                                         bias=sb["dec1b"][:])
                    pd2 = pb.tile([128, 512], F32, tag=f"ps2_{s_}")
                    nc.tensor.matmul(pd2[:, 0:256], sb["dec2W"][:], sd1[:],
                                     start=True, stop=True)
                    sd2 = ow.tile([128, OW], RD, tag=f"sd2{s_}")
                    nc.scalar.activation(sd2[:], pd2[:, 0:256], AF.Relu,
                                         bias=sb["dec2b"][:])
                    pd3 = pb.tile([128, 512], F32, tag=f"ps3_{s_}")
                    nc.tensor.matmul(pd3[0:2, 0:256], sb["dec3W"][:], sd2[:],
                                     start=True, stop=True)
                    yt = ow.tile([2, OW], F32, tag=f"y{s_}")
                    nc.vector.tensor_scalar(out=yt[:], in0=pd3[0:2, 0:256],
                                            scalar1=sb["dec3b"][:],
                                            scalar2=None, op0=ALU.add)
                    nc.sync.dma_start(y_out[:, sl], yt[:])

    if legalize:
        _legalize_matmul_waits(nc)
    return nc


def prep_inputs(inputs, T=T_FULL):
    """Host-side marshaling: shard x, build xt3/Wball layouts, repack weights.

    Scaling conventions (exact identities, see module docstring):
      - h is stored at half scale -> Whh and fc1_W pre-multiplied by 2
      - g-gate preactivation doubled -> g columns of Wih/Whh/bias x2
      - pn3 columns 1,2 negated so one Exp produces [Rp, Rd^-1, C^-1]
    """
    nxt = (T + SXT - 1) // SXT
    f = lambda a: np.ascontiguousarray(a, dtype=np.float32)
    x = f(inputs["x"])                      # [B, T, 2]
    Wih = f(inputs["lstm_Wih"])             # [2, 512]
    Whh = f(inputs["lstm_Whh"])             # [128, 512]
    bsum = f(inputs["lstm_bih"] + inputs["lstm_bhh"])   # [512]

    # permute gate chunks (i, f, g, o) -> (i, f, o, g)
    def permc(w):
        chunks = [w[..., 128 * cc : 128 * (cc + 1)] for cc in GATE_PERM]
        return np.concatenate(chunks, axis=-1)

    Wih_p, Whh_p, bsum_p = permc(Wih), permc(Whh), permc(bsum)

    # double the g-gate preactivation (tanh -> sigmoid trick)
    Wih_p = Wih_p.copy(); Whh_p = Whh_p.copy(); bsum_p = bsum_p.copy()
    Wih_p[:, 384:512] *= 2.0
    bsum_p[384:512] *= 2.0
    Whh_p[:, 384:512] *= 2.0
    # h stored at half scale
    Whh_p *= 2.0

    # Wball: [128, SXT*512]; slot s: rows 2s,2s+1 = Wih rows, row 32 = bias
    Wball = np.zeros((128, SXT * 512), dtype=np.float32)
    for s in range(SXT):
        Wball[2 * s, 512 * s : 512 * (s + 1)] = Wih_p[0]
        Wball[2 * s + 1, 512 * s : 512 * (s + 1)] = Wih_p[1]
        Wball[32, 512 * s : 512 * (s + 1)] = bsum_p

    # xt3 per core: [128, nxt*BP]; tile t//SXT, x rows 2(t%SXT), ones row 32
    xt3_all = np.zeros((NCORES, 128, nxt * BP), dtype=np.float32)
    xs = x.reshape(NCORES, BP, T, 2)
    for core in range(NCORES):
        xc = xs[core]                       # [BP, T, 2]
        for t in range(T):
            til, slot = divmod(t, SXT)
            col0 = BP * til
            xt3_all[core, 2 * slot, col0 : col0 + BP] = xc[:, t, 0]
            xt3_all[core, 2 * slot + 1, col0 : col0 + BP] = xc[:, t, 1]
        xt3_all[core, 32, :] = 1.0

    # selector rows for the ODE broadcasts: cols 0:128 -> rows0+rows1 (S_b),
    # cols 128:256 -> rows2 (C_b)
    erows = np.zeros((128, 384), dtype=np.float32)
    erows[0, 0:128] = 1.0
    erows[1, 0:128] = 1.0
    erows[2, 128:256] = 1.0

    def padw(w, rows, cols):
        out = np.zeros((rows, cols), dtype=np.float32)
        out[: w.shape[0], : w.shape[1]] = w
        return out

    def padb(b, rows):
        out = np.zeros((rows, 1), dtype=np.float32)
        out[: b.shape[0], 0] = b
        return out

    fc1_b = f(inputs["fc1_b"])
    fc2_W = f(inputs["fc2_W"])
    pn3_W = f(inputs["pn3_W"])              # [128, 3]
    pn3_b = f(inputs["pn3_b"])
    # negate cols 1,2 so exp([p0, -p1, -p2] + [b0, -b1, -b2]) gives
    # [Rp, Rd^-1, C^-1] in one activation
    pn3W_mod = pn3_W * np.array([1.0, -1.0, -1.0], dtype=np.float32)
    pn3bias3 = np.array([[pn3_b[0]], [-pn3_b[1]], [-pn3_b[2]]],
                        dtype=np.float32)
    dec1_W = f(inputs["dec1_W"])            # [131, 128]

    common = {
        "Wball": Wball,
        "Whh": f(Whh_p),
        "erows": erows,
        "fc1W": f(inputs["fc1_W"] * 2.0),
        "fc1b2": f(fc1_b.reshape(2, 128).T),
        "fc2W": f(np.concatenate([fc2_W[0:128], fc2_W[128:256]], axis=1)),
        "fc2b": f(inputs["fc2_b"][:, None]),
        "pn1W": padw(f(inputs["pn1_W"]), 128, 128),
        "pn1b": padb(f(inputs["pn1_b"]), 128),
        "pn2W": padw(f(inputs["pn2_W"]), 128, 128),
        "pn2b": f(inputs["pn2_b"][:, None]),
        "pn3W": f(pn3W_mod),
        "pn3Wpos": f(pn3_W[:, 1:3]),
        "pn3bias3": pn3bias3,
        "pn3biasB": np.array([[pn3_b[1]]], dtype=np.float32),
        "pn3biasC": np.array([[pn3_b[2]]], dtype=np.float32),
        "cn1W": padw(f(inputs["cn1_W"]), 128, 128),
        "cn1b": padb(f(inputs["cn1_b"]), 128),
        "cn2W": padw(f(inputs["cn2_W"]), 128, 128),
        "cn2b": f(inputs["cn2_b"][:, None]),
        "cn3W": f(inputs["cn3_W"]), "cn3b": f(inputs["cn3_b"][:, None]),
        "dec1aW": f(dec1_W[0:128]),
        "dec1b0W": f(dec1_W[128:129]), "dec1b1W": f(dec1_W[129:130]),
        "dec1b2W": f(dec1_W[130:131]),
        "dec1b": f(inputs["dec1_b"][:, None]),
        "dec2W": padw(f(inputs["dec2_W"]), 128, 128),
        "dec2b": padb(f(inputs["dec2_b"]), 128),
        "dec3W": padw(f(inputs["dec3_W"]), 128, 2),
        "dec3b": f(inputs["dec3_b"][:, None]),
    }
    aux = np.zeros((128, 128 + 2 * BS), dtype=np.float32)
    aux[:, 0:128] = 1.0
    common["aux"] = aux

    in_maps = []
    for core in range(NCORES):
        m = dict(common)
        m["xt3"] = xt3_all[core]
        in_maps.append(m)
    return in_maps


_PROGRAM = None


def get_program():
    global _PROGRAM
    if _PROGRAM is None:
        _PROGRAM = build_program()
    return _PROGRAM


def run(inputs, **kwargs):
    nc = get_program()
    in_maps = prep_inputs(inputs)
    res = run_bass_kernel_spmd(nc, in_maps, list(range(NCORES)), **kwargs)
    outs = [res.results[i]["y"] for i in range(NCORES)]   # each [2, BP]
    y = np.concatenate([o.T for o in outs], axis=0).astype(np.float32)  # [B, 2]
    return y, res


def kernel(**inputs):
    y, _ = run(inputs)
    return y


# revision 22
# speedup vs baseline: 1.3359x; 1.2904x over previous
"""Trainium2 Bass kernel for nn_BPModel: LSTM encoder -> latent ODE (RK4) -> decoder.

Data-parallel over 8 NeuronCores: batch 4096 -> 512 per core. All parameters
replicated. Everything stays on-chip (SBUF) in a transposed [feature, batch]
layout; matmuls run as fp32r (1 col/cycle at the PE when N>=256).

v2 redesign (vs v1 baseline at 1.85ms): the ACT (scalar) engine was the
bottleneck at 77% busy in the LSTM phase (8 sigmoid/tanh insts per step).
Key changes:

LSTM (T=256 steps, 2 interleaved half-batch streams of 256 cols each):
  - tanh eliminated via tanh(x) = 2*sigmoid(2x) - 1:
      * g-gate preactivation is pre-doubled (host scales the g columns of
        Wih/Whh/bias by 2), so one [128,1024] Sigmoid covers i|f|o|2g.
      * c-path: sigma2c = Sigmoid(c, scale=2) (ACT's free affine).
      * h is kept at half scale: h' = (sigma2c - 1/2)*sigma_o = h/2; the 2x
        is folded into Whh and fc1_W host-side. All identities exact.
  - per stream per step: 2 ACT insts (down from 4), 4 DVE insts.
  - gates PSUM: per stream one [128,1024] region [i|f|o|2g], pool bufs=2 so
    next step's x-projection matmuls run during this step's elementwise.
ODE: 9 fixed-grid Kutta-3/8 steps, 4 odef evals each, 2 interleaved
  half-batch streams (OSTR=2):
  - relu+bias folded into one tensor_scalar op (in+b max 0) on DVE/GPSIMD,
    keeping ACT nearly free for the exp.
  - pn3 as one [3,OW] matmul (cols 1,2 negated host-side) + one [3,OW] Exp.
  - Rp+Rd^-1 broadcast via one K=3 selector matmul; C^-1 via another.

Engine instructions carry a single HW sync-wait slot; a post-Tile pass
moves excess waits onto same-engine NoOps.
"""

import sys
import ml_dtypes
import numpy as np

for _p in ("/opt/trn_rl_repo",):
    if _p not in sys.path:
        sys.path.insert(0, _p)

import concourse.bass as bass
import concourse.tile as tile
import concourse.mybir as mybir
import concourse.bass_utils as _bu
from concourse.bass_utils import run_bass_kernel_spmd


def _patched_bir_verify_and_optimise(tmpdir, inp="bir.json", outp="file.neff",
                                     arch=None, *, dve_root=None):
    """Same as bass_utils.bir_verify_and_optimise but with walrus LDW
    dedup enabled (redundant LDWEIGHTS elision for back-to-back matmuls
    sharing a stationary operand)."""
    cmd = [
        _bu.get_walrus_driver(),
        "--pass",
        ",".join(["birverifier", "runtime_memory_reservation", "lower_act",
                  "lower_dve", "lower_ap_offset", "codegen", "neff_packager"]),
        "-i", inp,
        "--neff-output-filename", outp,
        "--enable-birsim=true", "--mem-mode=physical", "--policy=0",
        "--enable-ldw-opt=false",
        "--assign-static-dmas-to-sp=false",
        "--dram-page-size=256", "--enable-neff-debug-info=true",
        "--jobs", "8",
        *_bu.get_walrus_args(
            _bu.get_bir_arch(tmpdir, inp) if arch is None else arch,
            tmpdir, dve_root=dve_root),
    ]
    result = _bu.run_command(cmd, cwd=tmpdir)
    if result is not None:
        from pathlib import Path
        (Path(tmpdir) / "log.txt").write_text(result.stdout)
    return f"{tmpdir}/{outp}"


_bu.bir_verify_and_optimise = _patched_bir_verify_and_optimise

F32 = mybir.dt.float32
F32R = mybir.dt.float32r
BF16 = mybir.dt.bfloat16
AF = mybir.ActivationFunctionType
ALU = mybir.AluOpType

NCORES = 8
B, T_FULL, D_IN, H, LAT = 4096, 256, 2, 128, 128
BP = B // NCORES          # 512 batch per core
BS = BP // 2              # 256 per stream
N_STEPS = 9
SXT = 16                  # t-slots per xt3 tile (x rows 0..31, ones at 32)

# gate order in PSUM regions: i, f, o, g  (pytorch packs i, f, g, o)
GATE_PERM = (0, 1, 3, 2)

# weight tensors (fp32r tiles); bias tensors (f32 tiles)
# LSTM-path weights in bf16: bf16 matmuls run at the same 1 col/cycle but
# (unlike fp32r, which self-loads) support standalone LDWEIGHTS prefetch +
# walrus LDW elision, taking the weight load off the recurrence chain.
_BF16_W = {"Wball", "Whh", "fc1W", "fc2W"}
_W_SPECS = [
    ("Wball", [128, SXT * 512]),
    ("Whh", [128, 512]),
    ("erows", [128, 384]),
    ("fc1W", [128, 256]),
    ("fc2W", [128, 256]),
    ("pn1W", [128, 128]),
    ("pn2W", [128, 128]),
    ("pn3W", [128, 3]),
    ("pn3Wpos", [128, 2]),
    ("cn1W", [128, 128]),
    ("cn2W", [128, 128]),
    ("cn3W", [128, 128]),
    ("dec1aW", [128, 128]),
    ("dec1b0W", [1, 128]), ("dec1b1W", [1, 128]), ("dec1b2W", [1, 128]),
    ("dec2W", [128, 128]),
    ("dec3W", [128, 2]),
]
_B_SPECS = [
    ("fc1b2", [128, 2]),
    ("fc2b", [128, 1]),
    ("pn1b", [128, 1]), ("pn2b", [128, 1]),
    ("pn3bias3", [3, 1]), ("pn3biasB", [1, 1]), ("pn3biasC", [1, 1]),
    ("cn1b", [128, 1]), ("cn2b", [128, 1]), ("cn3b", [128, 1]),
    ("dec1b", [128, 1]),
    ("dec2b", [128, 1]),
    ("dec3b", [2, 1]),
]


def _f32(ap):
    return ap.bitcast(F32)


def _legalize_matmul_waits(nc):
    """Engine instructions carry a single HW sync-wait slot (walrus: 'Too
    many sync wait commands'). Move excess waits onto preceding NoOps on the
    same engine queue; engine FIFO order keeps correctness."""
    n_moved = 0
    for fn in nc.m.functions:
        for bb in fn.blocks:
            out = []
            for inst in bb.instructions:
                si = inst.sync_info
                if si is not None and si.on_wait and len(si.on_wait) > 1:
                    waits = list(si.on_wait)
                    for w in waits[:-1]:
                        nop = mybir.InstNoOp(
                            name=nc.get_next_instruction_name(),
                            engine=inst.engine,
                            ins=[], outs=[],
                            sync_info=mybir.SyncInfo(on_wait=[w], on_update=[]),
                        )
                        out.append(nop)
                    si.on_wait = waits[-1:]
                    n_moved += 1
                out.append(inst)
            bb.instructions[:] = out
    return n_moved


def build_program(T=T_FULL, n_steps=N_STEPS, use_f32r=True, debug=False,
                  legalize=True):
    RD = F32R if use_f32r else F32
    dt = 1.0 / n_steps
    nxt = (T + SXT - 1) // SXT
    nc = bass.Bass()
    ins = {}
    ins["xt3"] = nc.declare_dram_parameter("xt3", [128, nxt * BP], BF16,
                                           isOutput=False)
    for name, shape in _W_SPECS:
        wdt = BF16 if name in _BF16_W else RD
        ins[name] = nc.declare_dram_parameter(name, shape, wdt, isOutput=False)
    for name, shape in _B_SPECS:
        ins[name] = nc.declare_dram_parameter(name, shape, F32, isOutput=False)
    y_out = nc.declare_dram_parameter("y", [2, BP], F32, isOutput=True)
    if debug:
        dbg_h = nc.declare_dram_parameter("dbg_h", [128, BP], F32, isOutput=True)
        dbg_z0 = nc.declare_dram_parameter("dbg_z0", [128, BP], F32, isOutput=True)
        dbg_zT = nc.declare_dram_parameter("dbg_zT", [128, BP], F32, isOutput=True)
        dbg_pr = nc.declare_dram_parameter("dbg_pr", [3, BP], F32, isOutput=True)
        dbg_k = nc.declare_dram_parameter("dbg_k", [128, 4 * BP], F32,
                                          isOutput=True)

    with tile.TileContext(nc) as tc:
        with (
            tc.tile_pool(name="const", bufs=1) as cp,
            tc.tile_pool(name="state", bufs=2) as st,
        ):
            sb = {}
            sb["xt3"] = cp.tile([128, nxt * BP], BF16, tag="xt3", name="xt3")
            # split the 2MB xt3 DMA so step 0's x-projection only waits for
            # the first time-tile
            nc.sync.dma_start(sb["xt3"][:, 0:BP], ins["xt3"][:, 0:BP])
            nc.sync.dma_start(sb["xt3"][:, BP:], ins["xt3"][:, BP:])
            for name, shape in _W_SPECS:
                wdt = BF16 if name in _BF16_W else RD
                sb[name] = cp.tile(shape, wdt, tag=name, name=name)
                nc.sync.dma_start(sb[name][:], ins[name][:])
            for name, shape in _B_SPECS:
                sb[name] = cp.tile(shape, F32, tag=name, name=name)
                nc.sync.dma_start(sb[name][:], ins[name][:])
            paramA = cp.tile([1, BP], RD, tag="paramA")
            paramB = cp.tile([1, BP], RD, tag="paramB")
            paramC = cp.tile([1, BP], RD, tag="paramC")

            h = []
            c = []
            for s in range(2):
                ht = st.tile([128, BS], BF16, tag=f"h{s}")
                ct = st.tile([128, BS], BF16, tag=f"c{s}")
                nc.gpsimd.memset(ht[:], 0.0)
                nc.gpsimd.memset(ct[:], 0.0)
                h.append(ht)
                c.append(ct)

            xt3 = sb["xt3"]
            Wball = sb["Wball"]
            Whh = sb["Whh"]

            # ------------------ LSTM ------------------
            # per-stream gates psum [128,1024] = [i|f|o|2g], 256 cols each.
            # bufs=2: next step's x-projection matmuls (no h dependency) open
            # the other buffer's accumulation groups during this step's
            # elementwise chain.
            with (
                tc.tile_pool(name="psA", bufs=2, space="PSUM") as gp,
                tc.tile_pool(name="work", bufs=3) as wp,
            ):
                for t in range(T):
                    til, slot = divmod(t, SXT)
                    gates = {}
                    for s in range(2):
                        gates[s] = gp.tile([128, 1024], F32, tag=f"g{s}",
                                           name=f"g{s}_{t}")
                    # x-projection: 4 K=128 matmuls per stream (bias folded
                    # into Wball's ones row); ci-outer so consecutive matmuls
                    # share the stationary operand
                    for ci in range(4):
                        for s in range(2):
                            xsl = xt3[:, BP * til + BS * s
                                      : BP * til + BS * (s + 1)]
                            nc.tensor.matmul(
                                gates[s][:, 256 * ci : 256 * (ci + 1)],
                                Wball[:, 512 * slot + 128 * ci
                                      : 512 * slot + 128 * (ci + 1)],
                                xsl,
                                start=True, stop=False)
                    # recurrent part
                    for s in range(2):
                        for ci in range(4):
                            nc.tensor.matmul(
                                gates[s][:, 256 * ci : 256 * (ci + 1)],
                                Whh[:, 128 * ci : 128 * (ci + 1)],
                                h[s][:],
                                start=False, stop=True)
                    # one sigmoid over [i|f|o|2g]; sigma(2g) = (tanh(g)+1)/2
                    # bf16 outputs: DVE 2x mode on the elementwise chain
                    sgm = {}
                    for s in range(2):
                        sgm[s] = wp.tile([128, 1024], BF16, tag=f"sg{s}",
                                         name=f"sg{s}_{t}")
                        nc.scalar.activation(sgm[s][:], gates[s][:], AF.Sigmoid)
                    # c_new = 2*(sig2g - 1/2)*sig_i + sig_f*c
                    cn = {}
                    for s in range(2):
                        t1 = wp.tile([128, BS], BF16, tag=f"t1{s}", name=f"t1{s}_{t}")
                        nc.vector.scalar_tensor_tensor(
                            out=t1[:], in0=sgm[s][:, 768:1024], scalar=0.5,
                            in1=sgm[s][:, 0:256], op0=ALU.subtract, op1=ALU.mult)
                        t2 = wp.tile([128, BS], BF16, tag=f"t2{s}", name=f"t2{s}_{t}")
                        nc.vector.tensor_tensor(
                            out=t2[:], in0=sgm[s][:, 256:512], in1=c[s][:],
                            op=ALU.mult)
                        cn[s] = st.tile([128, BS], BF16, tag=f"c{s}", name=f"c{s}_{t}")
                        nc.vector.scalar_tensor_tensor(
                            out=cn[s][:], in0=t1[:], scalar=2.0, in1=t2[:],
                            op0=ALU.mult, op1=ALU.add)
                        c[s] = cn[s]
                    # sigma(2c) on ACT (free scale), then h' = (sig2c-1/2)*sig_o
                    sgc = {}
                    for s in range(2):
                        sgc[s] = wp.tile([128, BS], BF16, tag=f"tc{s}",
                                         name=f"tc{s}_{t}")
                        nc.scalar.activation(sgc[s][:], cn[s][:], AF.Sigmoid,
                                             scale=2.0)
                    for s in range(2):
                        hn_ = st.tile([128, BS], BF16, tag=f"h{s}", name=f"h{s}_{t}")
                        nc.vector.scalar_tensor_tensor(
                            out=hn_[:], in0=sgc[s][:], scalar=0.5,
                            in1=sgm[s][:, 512:768], op0=ALU.subtract,
                            op1=ALU.mult)
                        h[s] = hn_

            # ------------- encoder fc + ODE + decoder -------------
            OSTR = 2
            OW = BP // OSTR
            with (
                tc.tile_pool(name="psB", bufs=1, space="PSUM") as pb,
                tc.tile_pool(name="ow", bufs=2) as ow,
            ):
                if debug:
                    for s in range(2):
                        nc.sync.dma_start(
                            dbg_h[:, BS * s : BS * (s + 1)], _f32(h[s][:]))
                # fc1: hN @ fc1W + b -> relu ; chunks j of the 256-dim output
                # (fc1W pre-doubled host-side: h is at half scale)
                r1 = ow.tile([128, 1024], BF16, tag="r1")
                for j in range(2):
                    pfc = pb.tile([128, 512], F32, tag=f"ps1_{j}")
                    for s in range(2):
                        nc.tensor.matmul(
                            pfc[:, BS * s : BS * (s + 1)],
                            sb["fc1W"][:, 128 * j : 128 * (j + 1)],
                            h[s][:], start=True, stop=True)
                    nc.scalar.activation(
                        r1[:, 512 * j : 512 * (j + 1)], pfc[:], AF.Relu,
                        bias=sb["fc1b2"][:, j : j + 1])
                # fc2 (no relu)
                pz = pb.tile([128, BP], F32, tag="ps2_0")
                nc.tensor.matmul(pz[:], sb["fc2W"][:, 0:128], r1[:, 0:512],
                                 start=True, stop=False)
                nc.tensor.matmul(pz[:], sb["fc2W"][:, 128:256], r1[:, 512:1024],
                                 start=False, stop=True)
                zs = []
                for s_ in range(OSTR):
                    zt = ow.tile([128, OW], RD, tag=f"z{s_}")
                    nc.vector.tensor_scalar(
                        out=zt[:], in0=pz[:, OW * s_ : OW * (s_ + 1)],
                        scalar1=sb["fc2b"][:], scalar2=None, op0=ALU.add)
                    zs.append(zt)
                if debug:
                    for s_ in range(OSTR):
                        nc.sync.dma_start(dbg_z0[:, OW * s_ : OW * (s_ + 1)],
                                          _f32(zs[s_][:]))

                def relu_b(eng, out_ap, in_ap, bias_ap):
                    # out = max(in + b, 0) in one op on the given engine
                    eng.tensor_scalar(out=out_ap, in0=in_ap,
                                      scalar1=bias_ap, scalar2=0.0,
                                      op0=ALU.add, op1=ALU.max)

                def build_stream_ops(s_, z0t):
                    """Emit one stream's whole RK4 trajectory as a list of
                    stage-thunks; the driver interleaves the two streams with
                    a skew so their chains pipeline on the in-order engines."""
                    ops = []
                    env = {"z": z0t}
                    sl = slice(OW * s_, OW * (s_ + 1))

                    def odef_stages(zkey, kkey, first=False):
                        def st_l1():
                            p1 = pb.tile([128, 512], F32, tag=f"ps1_{s_}",
                                         name=f"p1_{s_}")
                            env["p1"] = p1
                            zin = env[zkey]
                            nc.tensor.matmul(p1[:, 0:256], sb["pn1W"][:],
                                             zin[:], start=True, stop=True)
                            nc.tensor.matmul(p1[:, 256:512], sb["cn1W"][:],
                                             zin[:], start=True, stop=True)
                        def st_r1():
                            s1 = ow.tile([128, 512], RD, tag=f"s1_{s_}",
                                         name=f"s1_{s_}")
                            env["s1"] = s1
                            relu_b(nc.vector, s1[:, 0:256],
                                   env["p1"][:, 0:256], sb["pn1b"][:])
                            nc.scalar.activation(s1[:, 256:512],
                                                 env["p1"][:, 256:512],
                                                 AF.Relu, bias=sb["cn1b"][:])
                        def st_l2():
                            p2 = pb.tile([128, 512], F32, tag=f"ps2_{s_}",
                                         name=f"p2_{s_}")
                            env["p2"] = p2
                            nc.tensor.matmul(p2[:, 0:256], sb["pn2W"][:],
                                             env["s1"][:, 0:256],
                                             start=True, stop=True)
                            nc.tensor.matmul(p2[:, 256:512], sb["cn2W"][:],
                                             env["s1"][:, 256:512],
                                             start=True, stop=True)
                        def st_r2():
                            s2 = ow.tile([128, 512], RD, tag=f"s2_{s_}",
                                         name=f"s2_{s_}")
                            env["s2"] = s2
                            relu_b(nc.vector, s2[:, 0:256],
                                   env["p2"][:, 0:256], sb["pn2b"][:])
                            nc.scalar.activation(s2[:, 256:512],
                                                 env["p2"][:, 256:512],
                                                 AF.Relu, bias=sb["cn2b"][:])
                        def st_l3():
                            p3 = pb.tile([128, 512], F32, tag=f"ps3_{s_}",
                                         name=f"p3_{s_}")
                            env["p3"] = p3
                            nc.tensor.matmul(p3[:, 0:256], sb["cn3W"][:],
                                             env["s2"][:, 256:512],
                                             start=True, stop=True)
                            nc.tensor.matmul(p3[0:3, 256:512], sb["pn3W"][:],
                                             env["s2"][:, 0:256],
                                             start=True, stop=True)
                        def st_exp():
                            rows = ow.tile([3, OW], RD, tag=f"rows{s_}",
                                           name=f"rows{s_}")
                            env["rows"] = rows
                            nc.scalar.activation(rows[:],
                                                 env["p3"][0:3, 256:512],
                                                 AF.Exp, bias=sb["pn3bias3"][:],
                                                 scale=1.0)
                        def st_params():
                            pp = pb.tile([128, 512], F32, tag=f"ps2_{s_}",
                                         name=f"pp{s_}")
                            nc.tensor.matmul(pp[0:1, 0:256],
                                             sb["pn3Wpos"][:, 0:1],
                                             env["s2"][:, 0:256],
                                             start=True, stop=True)
                            nc.tensor.matmul(pp[0:1, 256:512],
                                             sb["pn3Wpos"][:, 1:2],
                                             env["s2"][:, 0:256],
                                             start=True, stop=True)
                            nc.vector.tensor_copy(out=paramA[0:1, sl],
                                                  in_=env["rows"][0:1, :])
                            nc.scalar.activation(paramB[0:1, sl],
                                                 pp[0:1, 0:256], AF.Exp,
                                                 bias=sb["pn3biasB"][:],
                                                 scale=1.0)
                            nc.scalar.activation(paramC[0:1, sl],
                                                 pp[0:1, 256:512], AF.Exp,
                                                 bias=sb["pn3biasC"][:],
                                                 scale=1.0)
                        def st_bcast():
                            p4 = pb.tile([128, 512], F32, tag=f"ps4_{s_}",
                                         name=f"p4_{s_}")
                            env["p4"] = p4
                            nc.tensor.matmul(p4[:, 0:256],
                                             sb["erows"][0:3, 0:128],
                                             env["rows"][:],
                                             start=True, stop=True)
                            nc.tensor.matmul(p4[:, 256:512],
                                             sb["erows"][0:3, 128:256],
                                             env["rows"][:],
                                             start=True, stop=True)
                        def st_d1():
                            d1 = ow.tile([128, OW], F32, tag=f"d1{s_}",
                                         name=f"d1{s_}")
                            env["d1"] = d1
                            nc.vector.tensor_tensor(out=d1[:],
                                                    in0=_f32(env[zkey][:]),
                                                    in1=env["p4"][:, 0:256],
                                                    op=ALU.mult)
                        def st_d2():
                            d2 = ow.tile([128, OW], F32, tag=f"d2{s_}",
                                         name=f"d2{s_}")
                            env["d2"] = d2
                            nc.vector.scalar_tensor_tensor(
                                out=d2[:], in0=env["p3"][:, 0:256],
                                scalar=sb["cn3b"][:], in1=env["d1"][:],
                                op0=ALU.add, op1=ALU.subtract)
                        def st_k():
                            k = ow.tile([128, OW], F32, tag=f"{kkey}{s_}",
                                        name=f"{kkey}{s_}")
                            env[kkey] = k
                            nc.vector.tensor_tensor(out=k[:], in0=env["d2"][:],
                                                    in1=env["p4"][:, 256:512],
                                                    op=ALU.mult)
                        ops.extend([st_l1, st_r1, st_l2, st_r2, st_l3,
                                    st_exp])
                        if first:
                            ops.append(st_params)
                        ops.extend([st_bcast, st_d1, st_d2, st_k])

                    def glue(fn):
                        ops.append(fn)

                    def sttz1(kkey, scalar, zkey, okey):
                        def run():
                            o = ow.tile([128, OW], RD, tag=f"{okey}{s_}",
                                        name=f"{okey}{s_}")
                            nc.vector.scalar_tensor_tensor(
                                out=o[:], in0=env[kkey][:],
                                scalar=float(scalar), in1=_f32(env[zkey][:]),
                                op0=ALU.mult, op1=ALU.add)
                            env[okey] = o
                        return run

                    def sttk1(k0, scalar, k1_, okey):
                        def run():
                            o = ow.tile([128, OW], F32, tag=f"{okey}{s_}",
                                        name=f"{okey}{s_}")
                            nc.vector.scalar_tensor_tensor(
                                out=o[:], in0=env[k0][:], scalar=float(scalar),
                                in1=env[k1_][:], op0=ALU.mult, op1=ALU.add)
                            env[okey] = o
                        return run

                    def tt1(a, b, op, okey, eng=None):
                        def run():
                            o = ow.tile([128, OW], F32, tag=f"{okey}{s_}",
                                        name=f"{okey}{s_}")
                            (eng or nc.vector).tensor_tensor(
                                out=o[:], in0=env[a][:], in1=env[b][:], op=op)
                            env[okey] = o
                        return run

                    for step in range(n_steps):
                        odef_stages("z", "k1", first=(step == 0))
                        glue(sttz1("k1", dt / 3.0, "z", "za"))
                        odef_stages("za", "k2")
                        glue(sttk1("k1", -1.0 / 3.0, "k2", "u1"))
                        glue(sttz1("u1", dt, "z", "zb"))
                        odef_stages("zb", "k3")
                        glue(tt1("k1", "k2", ALU.subtract, "u1",
                                 eng=nc.gpsimd))
                        glue(tt1("u1", "k3", ALU.add, "u2", eng=nc.gpsimd))
                        glue(sttz1("u2", dt, "z", "zb"))
                        odef_stages("zb", "k4")
                        glue(tt1("k2", "k3", ALU.add, "u1", eng=nc.gpsimd))
                        glue(sttk1("u1", 3.0, "k1", "u2"))
                        glue(tt1("u2", "k4", ALU.add, "u1", eng=nc.gpsimd))
                        glue(sttz1("u1", dt / 8.0, "z", "z"))
                    return ops, env

                ops0, env0 = build_stream_ops(0, zs[0])
                ops1, env1 = build_stream_ops(1, zs[1])
                SKEW = 5
                n0, n1 = len(ops0), len(ops1)
                for i in range(max(n0, n1 + SKEW)):
                    if i < n0:
                        ops0[i]()
                    j = i - SKEW
                    if 0 <= j < n1:
                        ops1[j]()
                zs = [env0["z"], env1["z"]]

                for s_ in range(OSTR):
                    sl = slice(OW * s_, OW * (s_ + 1))
                    if debug:
                        nc.sync.dma_start(dbg_zT[:, sl], _f32(zs[s_][:]))
                        if s_ == 0:
                            nc.sync.dma_start(dbg_pr[0:1, :], _f32(paramA[:]))
                            nc.sync.dma_start(dbg_pr[1:2, :], _f32(paramB[:]))
                            nc.sync.dma_start(dbg_pr[2:3, :], _f32(paramC[:]))
                    # decoder: zc = [zT ; params]
                    pd1 = pb.tile([128, 512], F32, tag=f"ps1_{s_}")
                    nc.tensor.matmul(pd1[:, 0:256], sb["dec1aW"][:], zs[s_][:],
                                     start=True, stop=False)
                    nc.tensor.matmul(pd1[:, 0:256], sb["dec1b0W"][:],
                                     paramA[0:1, sl], start=False, stop=False)
                    nc.tensor.matmul(pd1[:, 0:256], sb["dec1b1W"][:],
                                     paramB[0:1, sl], start=False, stop=False)
                    nc.tensor.matmul(pd1[:, 0:256], sb["dec1b2W"][:],
                                     paramC[0:1, sl], start=False, stop=True)
                    sd1 = ow.tile([128, OW], RD, tag=f"sd1{s_}")
                    nc.scalar.activation(sd1[:], pd1[:, 0:256], AF.Relu,
                                         bias=sb["dec1b"][:])
                    pd2 = pb.tile([128, 512], F32, tag=f"ps2_{s_}")
                    nc.tensor.matmul(pd2[:, 0:256], sb["dec2W"][:], sd1[:],
                                     start=True, stop=True)
                    sd2 = ow.tile([128, OW], RD, tag=f"sd2{s_}")
                    nc.scalar.activation(sd2[:], pd2[:, 0:256], AF.Relu,
                                         bias=sb["dec2b"][:])
                    pd3 = pb.tile([128, 512], F32, tag=f"ps3_{s_}")
                    nc.tensor.matmul(pd3[0:2, 0:256], sb["dec3W"][:], sd2[:],
                                     start=True, stop=True)
                    yt = ow.tile([2, OW], F32, tag=f"y{s_}")
                    nc.vector.tensor_scalar(out=yt[:], in0=pd3[0:2, 0:256],
                                            scalar1=sb["dec3b"][:],
                                            scalar2=None, op0=ALU.add)
                    nc.sync.dma_start(y_out[:, sl], yt[:])

    if legalize:
        _legalize_matmul_waits(nc)
    return nc


def prep_inputs(inputs, T=T_FULL):
    """Host-side marshaling: shard x, build xt3/Wball layouts, repack weights.

    Scaling conventions (exact identities, see module docstring):
      - h is stored at half scale -> Whh and fc1_W pre-multiplied by 2
      - g-gate preactivation doubled -> g columns of Wih/Whh/bias x2
      - pn3 columns 1,2 negated so one Exp produces [Rp, Rd^-1, C^-1]
    """
    nxt = (T + SXT - 1) // SXT
    f = lambda a: np.ascontiguousarray(a, dtype=np.float32)
    x = f(inputs["x"])                      # [B, T, 2]
    Wih = f(inputs["lstm_Wih"])             # [2, 512]
    Whh = f(inputs["lstm_Whh"])             # [128, 512]
    bsum = f(inputs["lstm_bih"] + inputs["lstm_bhh"])   # [512]

    # permute gate chunks (i, f, g, o) -> (i, f, o, g)
    def permc(w):
        chunks = [w[..., 128 * cc : 128 * (cc + 1)] for cc in GATE_PERM]
        return np.concatenate(chunks, axis=-1)

    Wih_p, Whh_p, bsum_p = permc(Wih), permc(Whh), permc(bsum)

    # double the g-gate preactivation (tanh -> sigmoid trick)
    Wih_p = Wih_p.copy(); Whh_p = Whh_p.copy(); bsum_p = bsum_p.copy()
    Wih_p[:, 384:512] *= 2.0
    bsum_p[384:512] *= 2.0
    Whh_p[:, 384:512] *= 2.0
    # h stored at half scale
    Whh_p *= 2.0

    # Wball: [128, SXT*512]; slot s: rows 2s,2s+1 = Wih rows, row 32 = bias
    Wball = np.zeros((128, SXT * 512), dtype=np.float32)
    for s in range(SXT):
        Wball[2 * s, 512 * s : 512 * (s + 1)] = Wih_p[0]
        Wball[2 * s + 1, 512 * s : 512 * (s + 1)] = Wih_p[1]
        Wball[32, 512 * s : 512 * (s + 1)] = bsum_p

    # xt3 per core: [128, nxt*BP]; tile t//SXT, x rows 2(t%SXT), ones row 32
    xt3_all = np.zeros((NCORES, 128, nxt * BP), dtype=np.float32)
    xs = x.reshape(NCORES, BP, T, 2)
    for core in range(NCORES):
        xc = xs[core]                       # [BP, T, 2]
        for t in range(T):
            til, slot = divmod(t, SXT)
            col0 = BP * til
            xt3_all[core, 2 * slot, col0 : col0 + BP] = xc[:, t, 0]
            xt3_all[core, 2 * slot + 1, col0 : col0 + BP] = xc[:, t, 1]
        xt3_all[core, 32, :] = 1.0

    # selector rows for the ODE broadcasts: cols 0:128 -> rows0+rows1 (S_b),
    # cols 128:256 -> rows2 (C_b)
    erows = np.zeros((128, 384), dtype=np.float32)
    erows[0, 0:128] = 1.0
    erows[1, 0:128] = 1.0
    erows[2, 128:256] = 1.0

    def padw(w, rows, cols):
        out = np.zeros((rows, cols), dtype=np.float32)
        out[: w.shape[0], : w.shape[1]] = w
        return out

    def padb(b, rows):
        out = np.zeros((rows, 1), dtype=np.float32)
        out[: b.shape[0], 0] = b
        return out

    fc1_b = f(inputs["fc1_b"])
    fc2_W = f(inputs["fc2_W"])
    pn3_W = f(inputs["pn3_W"])              # [128, 3]
    pn3_b = f(inputs["pn3_b"])
    # negate cols 1,2 so exp([p0, -p1, -p2] + [b0, -b1, -b2]) gives
    # [Rp, Rd^-1, C^-1] in one activation
    pn3W_mod = pn3_W * np.array([1.0, -1.0, -1.0], dtype=np.float32)
    pn3bias3 = np.array([[pn3_b[0]], [-pn3_b[1]], [-pn3_b[2]]],
                        dtype=np.float32)
    dec1_W = f(inputs["dec1_W"])            # [131, 128]

    bf = lambda a: np.ascontiguousarray(a).astype(ml_dtypes.bfloat16)
    common = {
        "Wball": bf(Wball),
        "Whh": bf(Whh_p),
        "erows": erows,
        "fc1W": bf(inputs["fc1_W"] * 2.0),
        "fc1b2": f(fc1_b.reshape(2, 128).T),
        "fc2W": bf(np.concatenate([fc2_W[0:128], fc2_W[128:256]], axis=1)),
        "fc2b": f(inputs["fc2_b"][:, None]),
        "pn1W": padw(f(inputs["pn1_W"]), 128, 128),
        "pn1b": padb(f(inputs["pn1_b"]), 128),
        "pn2W": padw(f(inputs["pn2_W"]), 128, 128),
        "pn2b": f(inputs["pn2_b"][:, None]),
        "pn3W": f(pn3W_mod),
        "pn3Wpos": f(pn3_W[:, 1:3]),
        "pn3bias3": pn3bias3,
        "pn3biasB": np.array([[pn3_b[1]]], dtype=np.float32),
        "pn3biasC": np.array([[pn3_b[2]]], dtype=np.float32),
        "cn1W": padw(f(inputs["cn1_W"]), 128, 128),
        "cn1b": padb(f(inputs["cn1_b"]), 128),
        "cn2W": padw(f(inputs["cn2_W"]), 128, 128),
        "cn2b": f(inputs["cn2_b"][:, None]),
        "cn3W": f(inputs["cn3_W"]), "cn3b": f(inputs["cn3_b"][:, None]),
        "dec1aW": f(dec1_W[0:128]),
        "dec1b0W": f(dec1_W[128:129]), "dec1b1W": f(dec1_W[129:130]),
        "dec1b2W": f(dec1_W[130:131]),
        "dec1b": f(inputs["dec1_b"][:, None]),
        "dec2W": padw(f(inputs["dec2_W"]), 128, 128),
        "dec2b": padb(f(inputs["dec2_b"]), 128),
        "dec3W": padw(f(inputs["dec3_W"]), 128, 2),
        "dec3b": f(inputs["dec3_b"][:, None]),
    }
    xt3_bf = xt3_all.astype(ml_dtypes.bfloat16)
    in_maps = []
    for core in range(NCORES):
        m = dict(common)
        m["xt3"] = xt3_bf[core]
        in_maps.append(m)
    return in_maps


_PROGRAM = None


def get_program():
    global _PROGRAM
    if _PROGRAM is None:
        _PROGRAM = build_program()
    return _PROGRAM


def run(inputs, **kwargs):
    nc = get_program()
    in_maps = prep_inputs(inputs)
    res = run_bass_kernel_spmd(nc, in_maps, list(range(NCORES)), **kwargs)
    outs = [res.results[i]["y"] for i in range(NCORES)]   # each [2, BP]
    y = np.concatenate([o.T for o in outs], axis=0).astype(np.float32)  # [B, 2]
    return y, res


def kernel(**inputs):
    y, _ = run(inputs)
    return y


# revision 23
# speedup vs baseline: 1.3549x; 1.0142x over previous
"""Trainium2 Bass kernel for nn_BPModel: LSTM encoder -> latent ODE (RK4) -> decoder.

Data-parallel over 8 NeuronCores: batch 4096 -> 512 per core. All parameters
replicated. Everything stays on-chip (SBUF) in a transposed [feature, batch]
layout; matmuls run as fp32r (1 col/cycle at the PE when N>=256).

v2 redesign (vs v1 baseline at 1.85ms): the ACT (scalar) engine was the
bottleneck at 77% busy in the LSTM phase (8 sigmoid/tanh insts per step).
Key changes:

LSTM (T=256 steps, 2 interleaved half-batch streams of 256 cols each):
  - tanh eliminated via tanh(x) = 2*sigmoid(2x) - 1:
      * g-gate preactivation is pre-doubled (host scales the g columns of
        Wih/Whh/bias by 2), so one [128,1024] Sigmoid covers i|f|o|2g.
      * c-path: sigma2c = Sigmoid(c, scale=2) (ACT's free affine).
      * h is kept at half scale: h' = (sigma2c - 1/2)*sigma_o = h/2; the 2x
        is folded into Whh and fc1_W host-side. All identities exact.
  - per stream per step: 2 ACT insts (down from 4), 4 DVE insts.
  - gates PSUM: per stream one [128,1024] region [i|f|o|2g], pool bufs=2 so
    next step's x-projection matmuls run during this step's elementwise.
ODE: 9 fixed-grid Kutta-3/8 steps, 4 odef evals each, 2 interleaved
  half-batch streams (OSTR=2):
  - relu+bias folded into one tensor_scalar op (in+b max 0) on DVE/GPSIMD,
    keeping ACT nearly free for the exp.
  - pn3 as one [3,OW] matmul (cols 1,2 negated host-side) + one [3,OW] Exp.
  - Rp+Rd^-1 broadcast via one K=3 selector matmul; C^-1 via another.

Engine instructions carry a single HW sync-wait slot; a post-Tile pass
moves excess waits onto same-engine NoOps.
"""

import sys
import ml_dtypes
import numpy as np

for _p in ("/opt/trn_rl_repo",):
    if _p not in sys.path:
        sys.path.insert(0, _p)

import concourse.bass as bass
import concourse.tile as tile
import concourse.mybir as mybir
import concourse.bass_utils as _bu
from concourse.bass_utils import run_bass_kernel_spmd


def _patched_bir_verify_and_optimise(tmpdir, inp="bir.json", outp="file.neff",
                                     arch=None, *, dve_root=None):
    """Same as bass_utils.bir_verify_and_optimise but with walrus LDW
    dedup enabled (redundant LDWEIGHTS elision for back-to-back matmuls
    sharing a stationary operand)."""
    cmd = [
        _bu.get_walrus_driver(),
        "--pass",
        ",".join(["birverifier", "runtime_memory_reservation", "lower_act",
                  "lower_dve", "lower_ap_offset", "codegen", "neff_packager"]),
        "-i", inp,
        "--neff-output-filename", outp,
        "--enable-birsim=true", "--mem-mode=physical", "--policy=0",
        "--enable-ldw-opt=false",
        "--assign-static-dmas-to-sp=false",
        "--dram-page-size=256", "--enable-neff-debug-info=true",
        "--jobs", "8",
        *_bu.get_walrus_args(
            _bu.get_bir_arch(tmpdir, inp) if arch is None else arch,
            tmpdir, dve_root=dve_root),
    ]
    result = _bu.run_command(cmd, cwd=tmpdir)
    if result is not None:
        from pathlib import Path
        (Path(tmpdir) / "log.txt").write_text(result.stdout)
    return f"{tmpdir}/{outp}"


_bu.bir_verify_and_optimise = _patched_bir_verify_and_optimise

F32 = mybir.dt.float32
F32R = mybir.dt.float32r
BF16 = mybir.dt.bfloat16
AF = mybir.ActivationFunctionType
ALU = mybir.AluOpType

NCORES = 8
B, T_FULL, D_IN, H, LAT = 4096, 256, 2, 128, 128
BP = B // NCORES          # 512 batch per core
BS = BP // 2              # 256 per stream
N_STEPS = 9
SXT = 16                  # t-slots per xt3 tile (x rows 0..31, ones at 32)

# gate order in PSUM regions: i, f, o, g  (pytorch packs i, f, g, o)
GATE_PERM = (0, 1, 3, 2)

# weight tensors (fp32r tiles); bias tensors (f32 tiles)
# LSTM-path weights in bf16: bf16 matmuls run at the same 1 col/cycle but
# (unlike fp32r, which self-loads) support standalone LDWEIGHTS prefetch +
# walrus LDW elision, taking the weight load off the recurrence chain.
_BF16_W = {"Wball", "Whh", "fc1W", "fc2W", "erows", "pn1W", "pn2W",
           "pn3W", "pn3Wpos", "cn1W", "cn2W", "cn3W", "dec1aW",
           "dec1b0W", "dec1b1W", "dec1b2W", "dec2W", "dec3W"}
_W_SPECS = [
    ("Wball", [128, SXT * 512]),
    ("Whh", [128, 512]),
    ("erows", [128, 384]),
    ("fc1W", [128, 256]),
    ("fc2W", [128, 256]),
    ("pn1W", [128, 128]),
    ("pn2W", [128, 128]),
    ("pn3W", [128, 3]),
    ("pn3Wpos", [128, 2]),
    ("cn1W", [128, 128]),
    ("cn2W", [128, 128]),
    ("cn3W", [128, 128]),
    ("dec1aW", [128, 128]),
    ("dec1b0W", [1, 128]), ("dec1b1W", [1, 128]), ("dec1b2W", [1, 128]),
    ("dec2W", [128, 128]),
    ("dec3W", [128, 2]),
]
_B_SPECS = [
    ("fc1b2", [128, 2]),
    ("fc2b", [128, 1]),
    ("pn1b", [128, 1]), ("pn2b", [128, 1]),
    ("pn3bias3", [3, 1]), ("pn3biasB", [1, 1]), ("pn3biasC", [1, 1]),
    ("cn1b", [128, 1]), ("cn2b", [128, 1]), ("cn3b", [128, 1]),
    ("dec1b", [128, 1]),
    ("dec2b", [128, 1]),
    ("dec3b", [2, 1]),
]


def _f32(ap):
    return ap.bitcast(F32)


def _legalize_matmul_waits(nc):
    """Engine instructions carry a single HW sync-wait slot (walrus: 'Too
    many sync wait commands'). Move excess waits onto preceding NoOps on the
    same engine queue; engine FIFO order keeps correctness."""
    n_moved = 0
    for fn in nc.m.functions:
        for bb in fn.blocks:
            out = []
            for inst in bb.instructions:
                si = inst.sync_info
                if si is not None and si.on_wait and len(si.on_wait) > 1:
                    waits = list(si.on_wait)
                    for w in waits[:-1]:
                        nop = mybir.InstNoOp(
                            name=nc.get_next_instruction_name(),
                            engine=inst.engine,
                            ins=[], outs=[],
                            sync_info=mybir.SyncInfo(on_wait=[w], on_update=[]),
                        )
                        out.append(nop)
                    si.on_wait = waits[-1:]
                    n_moved += 1
                out.append(inst)
            bb.instructions[:] = out
    return n_moved


def build_program(T=T_FULL, n_steps=N_STEPS, use_f32r=True, debug=False,
                  legalize=True):
    RD = F32R if use_f32r else F32
    dt = 1.0 / n_steps
    nxt = (T + SXT - 1) // SXT
    nc = bass.Bass()
    ins = {}
    ins["xt3"] = nc.declare_dram_parameter("xt3", [128, nxt * BP], BF16,
                                           isOutput=False)
    for name, shape in _W_SPECS:
        wdt = BF16 if name in _BF16_W else RD
        ins[name] = nc.declare_dram_parameter(name, shape, wdt, isOutput=False)
    for name, shape in _B_SPECS:
        ins[name] = nc.declare_dram_parameter(name, shape, F32, isOutput=False)
    y_out = nc.declare_dram_parameter("y", [2, BP], F32, isOutput=True)
    if debug:
        dbg_h = nc.declare_dram_parameter("dbg_h", [128, BP], F32, isOutput=True)
        dbg_z0 = nc.declare_dram_parameter("dbg_z0", [128, BP], F32, isOutput=True)
        dbg_zT = nc.declare_dram_parameter("dbg_zT", [128, BP], F32, isOutput=True)
        dbg_pr = nc.declare_dram_parameter("dbg_pr", [3, BP], F32, isOutput=True)
        dbg_k = nc.declare_dram_parameter("dbg_k", [128, 4 * BP], F32,
                                          isOutput=True)

    with tile.TileContext(nc) as tc:
        with (
            tc.tile_pool(name="const", bufs=1) as cp,
            tc.tile_pool(name="state", bufs=2) as st,
        ):
            sb = {}
            sb["xt3"] = cp.tile([128, nxt * BP], BF16, tag="xt3", name="xt3")
            # split the 2MB xt3 DMA so step 0's x-projection only waits for
            # the first time-tile
            nc.sync.dma_start(sb["xt3"][:, 0:BP], ins["xt3"][:, 0:BP])
            nc.sync.dma_start(sb["xt3"][:, BP:], ins["xt3"][:, BP:])
            for name, shape in _W_SPECS:
                wdt = BF16 if name in _BF16_W else RD
                sb[name] = cp.tile(shape, wdt, tag=name, name=name)
                nc.sync.dma_start(sb[name][:], ins[name][:])
            for name, shape in _B_SPECS:
                sb[name] = cp.tile(shape, F32, tag=name, name=name)
                nc.sync.dma_start(sb[name][:], ins[name][:])
            paramA = cp.tile([1, BP], BF16, tag="paramA")
            paramB = cp.tile([1, BP], BF16, tag="paramB")
            paramC = cp.tile([1, BP], BF16, tag="paramC")

            h = []
            c = []
            for s in range(2):
                ht = st.tile([128, BS], BF16, tag=f"h{s}")
                ct = st.tile([128, BS], BF16, tag=f"c{s}")
                nc.gpsimd.memset(ht[:], 0.0)
                nc.gpsimd.memset(ct[:], 0.0)
                h.append(ht)
                c.append(ct)

            xt3 = sb["xt3"]
            Wball = sb["Wball"]
            Whh = sb["Whh"]

            # ------------------ LSTM ------------------
            # per-stream gates psum [128,1024] = [i|f|o|2g], 256 cols each.
            # bufs=2: next step's x-projection matmuls (no h dependency) open
            # the other buffer's accumulation groups during this step's
            # elementwise chain.
            with (
                tc.tile_pool(name="psA", bufs=2, space="PSUM") as gp,
                tc.tile_pool(name="work", bufs=3) as wp,
            ):
                for t in range(T):
                    til, slot = divmod(t, SXT)
                    gates = {}
                    for s in range(2):
                        gates[s] = gp.tile([128, 1024], F32, tag=f"g{s}",
                                           name=f"g{s}_{t}")
                    # x-projection: 4 K=128 matmuls per stream (bias folded
                    # into Wball's ones row); ci-outer so consecutive matmuls
                    # share the stationary operand
                    for ci in range(4):
                        for s in range(2):
                            xsl = xt3[:, BP * til + BS * s
                                      : BP * til + BS * (s + 1)]
                            nc.tensor.matmul(
                                gates[s][:, 256 * ci : 256 * (ci + 1)],
                                Wball[:, 512 * slot + 128 * ci
                                      : 512 * slot + 128 * (ci + 1)],
                                xsl,
                                start=True, stop=False)
                    # recurrent part
                    for s in range(2):
                        for ci in range(4):
                            nc.tensor.matmul(
                                gates[s][:, 256 * ci : 256 * (ci + 1)],
                                Whh[:, 128 * ci : 128 * (ci + 1)],
                                h[s][:],
                                start=False, stop=True)
                    # one sigmoid over [i|f|o|2g]; sigma(2g) = (tanh(g)+1)/2
                    # bf16 outputs: DVE 2x mode on the elementwise chain
                    sgm = {}
                    for s in range(2):
                        sgm[s] = wp.tile([128, 1024], BF16, tag=f"sg{s}",
                                         name=f"sg{s}_{t}")
                        nc.scalar.activation(sgm[s][:], gates[s][:], AF.Sigmoid)
                    # c_new = 2*(sig2g - 1/2)*sig_i + sig_f*c
                    cn = {}
                    for s in range(2):
                        t1 = wp.tile([128, BS], BF16, tag=f"t1{s}", name=f"t1{s}_{t}")
                        nc.vector.scalar_tensor_tensor(
                            out=t1[:], in0=sgm[s][:, 768:1024], scalar=0.5,
                            in1=sgm[s][:, 0:256], op0=ALU.subtract, op1=ALU.mult)
                        t2 = wp.tile([128, BS], BF16, tag=f"t2{s}", name=f"t2{s}_{t}")
                        nc.vector.tensor_tensor(
                            out=t2[:], in0=sgm[s][:, 256:512], in1=c[s][:],
                            op=ALU.mult)
                        cn[s] = st.tile([128, BS], BF16, tag=f"c{s}", name=f"c{s}_{t}")
                        nc.vector.scalar_tensor_tensor(
                            out=cn[s][:], in0=t1[:], scalar=2.0, in1=t2[:],
                            op0=ALU.mult, op1=ALU.add)
                        c[s] = cn[s]
                    # sigma(2c) on ACT (free scale), then h' = (sig2c-1/2)*sig_o
                    sgc = {}
                    for s in range(2):
                        sgc[s] = wp.tile([128, BS], BF16, tag=f"tc{s}",
                                         name=f"tc{s}_{t}")
                        nc.scalar.activation(sgc[s][:], cn[s][:], AF.Sigmoid,
                                             scale=2.0)
                    for s in range(2):
                        hn_ = st.tile([128, BS], BF16, tag=f"h{s}", name=f"h{s}_{t}")
                        nc.vector.scalar_tensor_tensor(
                            out=hn_[:], in0=sgc[s][:], scalar=0.5,
                            in1=sgm[s][:, 512:768], op0=ALU.subtract,
                            op1=ALU.mult)
                        h[s] = hn_

            # ------------- encoder fc + ODE + decoder -------------
            OSTR = 2
            OW = BP // OSTR
            with (
                tc.tile_pool(name="psB", bufs=1, space="PSUM") as pb,
                tc.tile_pool(name="ow", bufs=2) as ow,
            ):
                if debug:
                    for s in range(2):
                        nc.sync.dma_start(
                            dbg_h[:, BS * s : BS * (s + 1)], _f32(h[s][:]))
                # fc1: hN @ fc1W + b -> relu ; chunks j of the 256-dim output
                # (fc1W pre-doubled host-side: h is at half scale)
                r1 = ow.tile([128, 1024], BF16, tag="r1")
                for j in range(2):
                    pfc = pb.tile([128, 512], F32, tag=f"ps1_{j}")
                    for s in range(2):
                        nc.tensor.matmul(
                            pfc[:, BS * s : BS * (s + 1)],
                            sb["fc1W"][:, 128 * j : 128 * (j + 1)],
                            h[s][:], start=True, stop=True)
                    nc.scalar.activation(
                        r1[:, 512 * j : 512 * (j + 1)], pfc[:], AF.Relu,
                        bias=sb["fc1b2"][:, j : j + 1])
                # fc2 (no relu)
                pz = pb.tile([128, BP], F32, tag="ps2_0")
                nc.tensor.matmul(pz[:], sb["fc2W"][:, 0:128], r1[:, 0:512],
                                 start=True, stop=False)
                nc.tensor.matmul(pz[:], sb["fc2W"][:, 128:256], r1[:, 512:1024],
                                 start=False, stop=True)
                zs = []
                for s_ in range(OSTR):
                    zt = ow.tile([128, OW], BF16, tag=f"z{s_}")
                    nc.vector.tensor_scalar(
                        out=zt[:], in0=pz[:, OW * s_ : OW * (s_ + 1)],
                        scalar1=sb["fc2b"][:], scalar2=None, op0=ALU.add)
                    zs.append(zt)
                if debug:
                    for s_ in range(OSTR):
                        nc.sync.dma_start(dbg_z0[:, OW * s_ : OW * (s_ + 1)],
                                          _f32(zs[s_][:]))

                def relu_b(eng, out_ap, in_ap, bias_ap):
                    # out = max(in + b, 0) in one op on the given engine
                    eng.tensor_scalar(out=out_ap, in0=in_ap,
                                      scalar1=bias_ap, scalar2=0.0,
                                      op0=ALU.add, op1=ALU.max)

                def build_stream_ops(s_, z0t):
                    """Emit one stream's whole RK4 trajectory as a list of
                    stage-thunks; the driver interleaves the two streams with
                    a skew so their chains pipeline on the in-order engines."""
                    ops = []
                    env = {"z": z0t}
                    sl = slice(OW * s_, OW * (s_ + 1))

                    def odef_stages(zkey, kkey, first=False):
                        def st_l1():
                            p1 = pb.tile([128, 512], F32, tag=f"ps1_{s_}",
                                         name=f"p1_{s_}")
                            env["p1"] = p1
                            zin = env[zkey]
                            nc.tensor.matmul(p1[:, 0:256], sb["pn1W"][:],
                                             zin[:], start=True, stop=True)
                            nc.tensor.matmul(p1[:, 256:512], sb["cn1W"][:],
                                             zin[:], start=True, stop=True)
                        def st_r1():
                            s1 = ow.tile([128, 512], BF16, tag=f"s1_{s_}",
                                         name=f"s1_{s_}")
                            env["s1"] = s1
                            relu_b(nc.vector, s1[:, 0:256],
                                   env["p1"][:, 0:256], sb["pn1b"][:])
                            nc.scalar.activation(s1[:, 256:512],
                                                 env["p1"][:, 256:512],
                                                 AF.Relu, bias=sb["cn1b"][:])
                        def st_l2():
                            p2 = pb.tile([128, 512], F32, tag=f"ps2_{s_}",
                                         name=f"p2_{s_}")
                            env["p2"] = p2
                            nc.tensor.matmul(p2[:, 0:256], sb["pn2W"][:],
                                             env["s1"][:, 0:256],
                                             start=True, stop=True)
                            nc.tensor.matmul(p2[:, 256:512], sb["cn2W"][:],
                                             env["s1"][:, 256:512],
                                             start=True, stop=True)
                        def st_r2():
                            s2 = ow.tile([128, 512], BF16, tag=f"s2_{s_}",
                                         name=f"s2_{s_}")
                            env["s2"] = s2
                            relu_b(nc.vector, s2[:, 0:256],
                                   env["p2"][:, 0:256], sb["pn2b"][:])
                            nc.scalar.activation(s2[:, 256:512],
                                                 env["p2"][:, 256:512],
                                                 AF.Relu, bias=sb["cn2b"][:])
                        def st_l3():
                            p3 = pb.tile([128, 512], F32, tag=f"ps3_{s_}",
                                         name=f"p3_{s_}")
                            env["p3"] = p3
                            nc.tensor.matmul(p3[:, 0:256], sb["cn3W"][:],
                                             env["s2"][:, 256:512],
                                             start=True, stop=True)
                            nc.tensor.matmul(p3[0:3, 256:512], sb["pn3W"][:],
                                             env["s2"][:, 0:256],
                                             start=True, stop=True)
                        def st_exp():
                            rows = ow.tile([3, OW], BF16, tag=f"rows{s_}",
                                           name=f"rows{s_}")
                            env["rows"] = rows
                            nc.scalar.activation(rows[:],
                                                 env["p3"][0:3, 256:512],
                                                 AF.Exp, bias=sb["pn3bias3"][:],
                                                 scale=1.0)
                        def st_params():
                            pp = pb.tile([128, 512], F32, tag=f"ps2_{s_}",
                                         name=f"pp{s_}")
                            nc.tensor.matmul(pp[0:1, 0:256],
                                             sb["pn3Wpos"][:, 0:1],
                                             env["s2"][:, 0:256],
                                             start=True, stop=True)
                            nc.tensor.matmul(pp[0:1, 256:512],
                                             sb["pn3Wpos"][:, 1:2],
                                             env["s2"][:, 0:256],
                                             start=True, stop=True)
                            nc.vector.tensor_copy(out=paramA[0:1, sl],
                                                  in_=env["rows"][0:1, :])
                            nc.scalar.activation(paramB[0:1, sl],
                                                 pp[0:1, 0:256], AF.Exp,
                                                 bias=sb["pn3biasB"][:],
                                                 scale=1.0)
                            nc.scalar.activation(paramC[0:1, sl],
                                                 pp[0:1, 256:512], AF.Exp,
                                                 bias=sb["pn3biasC"][:],
                                                 scale=1.0)
                        def st_bcast():
                            p4 = pb.tile([128, 512], F32, tag=f"ps4_{s_}",
                                         name=f"p4_{s_}")
                            env["p4"] = p4
                            nc.tensor.matmul(p4[:, 0:256],
                                             sb["erows"][0:3, 0:128],
                                             env["rows"][:],
                                             start=True, stop=True)
                            nc.tensor.matmul(p4[:, 256:512],
                                             sb["erows"][0:3, 128:256],
                                             env["rows"][:],
                                             start=True, stop=True)
                        def st_d1():
                            d1 = ow.tile([128, OW], F32, tag=f"d1{s_}",
                                         name=f"d1{s_}")
                            env["d1"] = d1
                            nc.vector.tensor_tensor(out=d1[:],
                                                    in0=env[zkey][:],
                                                    in1=env["p4"][:, 0:256],
                                                    op=ALU.mult)
                        def st_d2():
                            d2 = ow.tile([128, OW], F32, tag=f"d2{s_}",
                                         name=f"d2{s_}")
                            env["d2"] = d2
                            nc.vector.scalar_tensor_tensor(
                                out=d2[:], in0=env["p3"][:, 0:256],
                                scalar=sb["cn3b"][:], in1=env["d1"][:],
                                op0=ALU.add, op1=ALU.subtract)
                        def st_k():
                            k = ow.tile([128, OW], F32, tag=f"{kkey}{s_}",
                                        name=f"{kkey}{s_}")
                            env[kkey] = k
                            nc.vector.tensor_tensor(out=k[:], in0=env["d2"][:],
                                                    in1=env["p4"][:, 256:512],
                                                    op=ALU.mult)
                        ops.extend([st_l1, st_r1, st_l2, st_r2, st_l3,
                                    st_exp])
                        if first:
                            ops.append(st_params)
                        ops.extend([st_bcast, st_d1, st_d2, st_k])

                    def glue(fn):
                        ops.append(fn)

                    def sttz1(kkey, scalar, zkey, okey):
                        def run():
                            o = ow.tile([128, OW], BF16, tag=f"{okey}{s_}",
                                        name=f"{okey}{s_}")
                            nc.vector.scalar_tensor_tensor(
                                out=o[:], in0=env[kkey][:],
                                scalar=float(scalar), in1=env[zkey][:],
                                op0=ALU.mult, op1=ALU.add)
                            env[okey] = o
                        return run

                    def sttk1(k0, scalar, k1_, okey):
                        def run():
                            o = ow.tile([128, OW], F32, tag=f"{okey}{s_}",
                                        name=f"{okey}{s_}")
                            nc.vector.scalar_tensor_tensor(
                                out=o[:], in0=env[k0][:], scalar=float(scalar),
                                in1=env[k1_][:], op0=ALU.mult, op1=ALU.add)
                            env[okey] = o
                        return run

                    def tt1(a, b, op, okey, eng=None):
                        def run():
                            o = ow.tile([128, OW], F32, tag=f"{okey}{s_}",
                                        name=f"{okey}{s_}")
                            (eng or nc.vector).tensor_tensor(
                                out=o[:], in0=env[a][:], in1=env[b][:], op=op)
                            env[okey] = o
                        return run

                    for step in range(n_steps):
                        odef_stages("z", "k1", first=(step == 0))
                        glue(sttz1("k1", dt / 3.0, "z", "za"))
                        odef_stages("za", "k2")
                        glue(sttk1("k1", -1.0 / 3.0, "k2", "u1"))
                        glue(sttz1("u1", dt, "z", "zb"))
                        odef_stages("zb", "k3")
                        glue(tt1("k1", "k2", ALU.subtract, "u1",
                                 eng=nc.gpsimd))
                        glue(tt1("u1", "k3", ALU.add, "u2", eng=nc.gpsimd))
                        glue(sttz1("u2", dt, "z", "zb"))
                        odef_stages("zb", "k4")
                        glue(tt1("k2", "k3", ALU.add, "u1", eng=nc.gpsimd))
                        glue(sttk1("u1", 3.0, "k1", "u2"))
                        glue(tt1("u2", "k4", ALU.add, "u1", eng=nc.gpsimd))
                        glue(sttz1("u1", dt / 8.0, "z", "z"))
                    return ops, env

                ops0, env0 = build_stream_ops(0, zs[0])
                ops1, env1 = build_stream_ops(1, zs[1])
                SKEW = 5
                n0, n1 = len(ops0), len(ops1)
                for i in range(max(n0, n1 + SKEW)):
                    if i < n0:
                        ops0[i]()
                    j = i - SKEW
                    if 0 <= j < n1:
                        ops1[j]()
                zs = [env0["z"], env1["z"]]

                for s_ in range(OSTR):
                    sl = slice(OW * s_, OW * (s_ + 1))
                    if debug:
                        nc.sync.dma_start(dbg_zT[:, sl], _f32(zs[s_][:]))
                        if s_ == 0:
                            nc.sync.dma_start(dbg_pr[0:1, :], _f32(paramA[:]))
                            nc.sync.dma_start(dbg_pr[1:2, :], _f32(paramB[:]))
                            nc.sync.dma_start(dbg_pr[2:3, :], _f32(paramC[:]))
                    # decoder: zc = [zT ; params]
                    pd1 = pb.tile([128, 512], F32, tag=f"ps1_{s_}")
                    nc.tensor.matmul(pd1[:, 0:256], sb["dec1aW"][:], zs[s_][:],
                                     start=True, stop=False)
                    nc.tensor.matmul(pd1[:, 0:256], sb["dec1b0W"][:],
                                     paramA[0:1, sl], start=False, stop=False)
                    nc.tensor.matmul(pd1[:, 0:256], sb["dec1b1W"][:],
                                     paramB[0:1, sl], start=False, stop=False)
                    nc.tensor.matmul(pd1[:, 0:256], sb["dec1b2W"][:],
                                     paramC[0:1, sl], start=False, stop=True)
                    sd1 = ow.tile([128, OW], BF16, tag=f"sd1{s_}")
                    nc.scalar.activation(sd1[:], pd1[:, 0:256], AF.Relu,
                                         bias=sb["dec1b"][:])
                    pd2 = pb.tile([128, 512], F32, tag=f"ps2_{s_}")
                    nc.tensor.matmul(pd2[:, 0:256], sb["dec2W"][:], sd1[:],
                                     start=True, stop=True)
                    sd2 = ow.tile([128, OW], BF16, tag=f"sd2{s_}")
                    nc.scalar.activation(sd2[:], pd2[:, 0:256], AF.Relu,
                                         bias=sb["dec2b"][:])
                    pd3 = pb.tile([128, 512], F32, tag=f"ps3_{s_}")
                    nc.tensor.matmul(pd3[0:2, 0:256], sb["dec3W"][:], sd2[:],
                                     start=True, stop=True)
                    yt = ow.tile([2, OW], F32, tag=f"y{s_}")
                    nc.vector.tensor_scalar(out=yt[:], in0=pd3[0:2, 0:256],
                                            scalar1=sb["dec3b"][:],
                                            scalar2=None, op0=ALU.add)
                    nc.sync.dma_start(y_out[:, sl], yt[:])

    if legalize:
        _legalize_matmul_waits(nc)
    return nc


def prep_inputs(inputs, T=T_FULL):
    """Host-side marshaling: shard x, build xt3/Wball layouts, repack weights.

    Scaling conventions (exact identities, see module docstring):
      - h is stored at half scale -> Whh and fc1_W pre-multiplied by 2
      - g-gate preactivation doubled -> g columns of Wih/Whh/bias x2
      - pn3 columns 1,2 negated so one Exp produces [Rp, Rd^-1, C^-1]
    """
    nxt = (T + SXT - 1) // SXT
    f = lambda a: np.ascontiguousarray(a, dtype=np.float32)
    x = f(inputs["x"])                      # [B, T, 2]
    Wih = f(inputs["lstm_Wih"])             # [2, 512]
    Whh = f(inputs["lstm_Whh"])             # [128, 512]
    bsum = f(inputs["lstm_bih"] + inputs["lstm_bhh"])   # [512]

    # permute gate chunks (i, f, g, o) -> (i, f, o, g)
    def permc(w):
        chunks = [w[..., 128 * cc : 128 * (cc + 1)] for cc in GATE_PERM]
        return np.concatenate(chunks, axis=-1)

    Wih_p, Whh_p, bsum_p = permc(Wih), permc(Whh), permc(bsum)

    # double the g-gate preactivation (tanh -> sigmoid trick)
    Wih_p = Wih_p.copy(); Whh_p = Whh_p.copy(); bsum_p = bsum_p.copy()
    Wih_p[:, 384:512] *= 2.0
    bsum_p[384:512] *= 2.0
    Whh_p[:, 384:512] *= 2.0
    # h stored at half scale
    Whh_p *= 2.0

    # Wball: [128, SXT*512]; slot s: rows 2s,2s+1 = Wih rows, row 32 = bias
    Wball = np.zeros((128, SXT * 512), dtype=np.float32)
    for s in range(SXT):
        Wball[2 * s, 512 * s : 512 * (s + 1)] = Wih_p[0]
        Wball[2 * s + 1, 512 * s : 512 * (s + 1)] = Wih_p[1]
        Wball[32, 512 * s : 512 * (s + 1)] = bsum_p

    # xt3 per core: [128, nxt*BP]; tile t//SXT, x rows 2(t%SXT), ones row 32
    xt3_all = np.zeros((NCORES, 128, nxt * BP), dtype=np.float32)
    xs = x.reshape(NCORES, BP, T, 2)
    for core in range(NCORES):
        xc = xs[core]                       # [BP, T, 2]
        for t in range(T):
            til, slot = divmod(t, SXT)
            col0 = BP * til
            xt3_all[core, 2 * slot, col0 : col0 + BP] = xc[:, t, 0]
            xt3_all[core, 2 * slot + 1, col0 : col0 + BP] = xc[:, t, 1]
        xt3_all[core, 32, :] = 1.0

    # selector rows for the ODE broadcasts: cols 0:128 -> rows0+rows1 (S_b),
    # cols 128:256 -> rows2 (C_b)
    erows = np.zeros((128, 384), dtype=np.float32)
    erows[0, 0:128] = 1.0
    erows[1, 0:128] = 1.0
    erows[2, 128:256] = 1.0

    def padw(w, rows, cols):
        out = np.zeros((rows, cols), dtype=np.float32)
        out[: w.shape[0], : w.shape[1]] = w
        return out

    def padb(b, rows):
        out = np.zeros((rows, 1), dtype=np.float32)
        out[: b.shape[0], 0] = b
        return out

    fc1_b = f(inputs["fc1_b"])
    fc2_W = f(inputs["fc2_W"])
    pn3_W = f(inputs["pn3_W"])              # [128, 3]
    pn3_b = f(inputs["pn3_b"])
    # negate cols 1,2 so exp([p0, -p1, -p2] + [b0, -b1, -b2]) gives
    # [Rp, Rd^-1, C^-1] in one activation
    pn3W_mod = pn3_W * np.array([1.0, -1.0, -1.0], dtype=np.float32)
    pn3bias3 = np.array([[pn3_b[0]], [-pn3_b[1]], [-pn3_b[2]]],
                        dtype=np.float32)
    dec1_W = f(inputs["dec1_W"])            # [131, 128]

    bf = lambda a: np.ascontiguousarray(a).astype(ml_dtypes.bfloat16)
    common = {
        "Wball": bf(Wball),
        "Whh": bf(Whh_p),
        "erows": bf(erows),
        "fc1W": bf(inputs["fc1_W"] * 2.0),
        "fc1b2": f(fc1_b.reshape(2, 128).T),
        "fc2W": bf(np.concatenate([fc2_W[0:128], fc2_W[128:256]], axis=1)),
        "fc2b": f(inputs["fc2_b"][:, None]),
        "pn1W": bf(padw(f(inputs["pn1_W"]), 128, 128)),
        "pn1b": padb(f(inputs["pn1_b"]), 128),
        "pn2W": bf(padw(f(inputs["pn2_W"]), 128, 128)),
        "pn2b": f(inputs["pn2_b"][:, None]),
        "pn3W": bf(pn3W_mod),
        "pn3Wpos": bf(pn3_W[:, 1:3]),
        "pn3bias3": pn3bias3,
        "pn3biasB": np.array([[pn3_b[1]]], dtype=np.float32),
        "pn3biasC": np.array([[pn3_b[2]]], dtype=np.float32),
        "cn1W": bf(padw(f(inputs["cn1_W"]), 128, 128)),
        "cn1b": padb(f(inputs["cn1_b"]), 128),
        "cn2W": bf(padw(f(inputs["cn2_W"]), 128, 128)),
        "cn2b": f(inputs["cn2_b"][:, None]),
        "cn3W": bf(f(inputs["cn3_W"])), "cn3b": f(inputs["cn3_b"][:, None]),
        "dec1aW": bf(dec1_W[0:128]),
        "dec1b0W": bf(dec1_W[128:129]), "dec1b1W": bf(dec1_W[129:130]),
        "dec1b2W": bf(dec1_W[130:131]),
        "dec1b": f(inputs["dec1_b"][:, None]),
        "dec2W": bf(padw(f(inputs["dec2_W"]), 128, 128)),
        "dec2b": padb(f(inputs["dec2_b"]), 128),
        "dec3W": bf(padw(f(inputs["dec3_W"]), 128, 2)),
        "dec3b": f(inputs["dec3_b"][:, None]),
    }
    xt3_bf = xt3_all.astype(ml_dtypes.bfloat16)
    in_maps = []
    for core in range(NCORES):
        m = dict(common)
        m["xt3"] = xt3_bf[core]
        in_maps.append(m)
    return in_maps


_PROGRAM = None


def get_program():
    global _PROGRAM
    if _PROGRAM is None:
        _PROGRAM = build_program()
    return _PROGRAM


def run(inputs, **kwargs):
    nc = get_program()
    in_maps = prep_inputs(inputs)
    res = run_bass_kernel_spmd(nc, in_maps, list(range(NCORES)), **kwargs)
    outs = [res.results[i]["y"] for i in range(NCORES)]   # each [2, BP]
    y = np.concatenate([o.T for o in outs], axis=0).astype(np.float32)  # [B, 2]
    return y, res


def kernel(**inputs):
    y, _ = run(inputs)
    return y
